# revision 10
# baseline (speedup 1.0000x reference)
"""2-layer GAT (edge features, softmax attention over dst, max aggregation)
on 8 TRN2 NeuronCores — dst-sharded, edge-slot streaming formulation.

Host: sorts edges by dst, assigns dst nodes to the 8 cores round-robin by
degree rank (identical SPMD tile structure on every core), and packs per-edge
operands into dense [82, S] bf16 streams (per-node runs of padded degree d_t
along the free axis). Row 80 of the stream carries the precomputed
ad[dst] = x_dst . (W @ a_d) term so the attention logit is one matmul.

Device per tile pair (two 64-row halves sharing a [128, w] PSUM tile):
PE computes messages h[src]+e (lmsg) and 64x-replicated attention logits
(llog, includes as/ae/ad/pad rows); ACT applies leaky-relu in place on the
logits then exp -> p (bf16); DVE multiplies messages by p and does segmented
max (messages) / sum (p) reduces over the per-node runs. Equal-shape pair
quads share [128, 2, w] PSUM tiles so ACT/DVE ops run at FD=2w, halving
per-instruction overhead. Softmax division, +b, final leaky-relu and
empty-segment fixup happen once per layer on [128, NCOL] accumulators.
The inter-layer gather c1[src] is a host-side data reshuffle between two
launches of one compiled program.

Numerics: pad slots get logit += PAD_LOGIT (p~1e-13, vanishes in sums) and
message value BIG_NEG (never wins max). Softmax max-subtraction is dropped
(exact softmax invariance; |logits| << 80 so exp cannot overflow). Division
by the positive per-node softmax sum commutes with max, so it is applied
post-reduce.
"""

import os
import numpy as np
import ml_dtypes
from contextlib import ExitStack

import concourse.bacc as bacc
import concourse.bass as bass
import concourse.mybir as mybir
import concourse.tile as tile
from concourse.bass_utils import run_bass_kernel_spmd

N = 50000
E = 1600000
DIN = 64
DOUT = 64
DE = 16
NC = 8
NPC = N // NC
ATT_SLOPE = 0.2
ACT_SLOPE = 0.01
PAD_LOGIT = -150.0
BIG_NEG = -1.0e30
EMPTY_THR = -1.0e6
ROW_EA = DIN            # 64..79: edge attr
ROW_AD = DIN + DE       # 80: host-computed ad[dst]
ROW_PAD = DIN + DE + 1  # 81: pad flag
K_RHS = DIN + DE + 2    # 82
CHUNK_COLS = 8192
TILE_W = 512

LAST_EXEC_NS = []

_bf16 = mybir.dt.bfloat16
_f32 = mybir.dt.float32


def _bf(a):
    return np.asarray(a, np.float32).astype(ml_dtypes.bfloat16)


def _install_ntff_shim():
    """Register the axon NTFF profiling hook so trace=True returns HW exec
    times. Best-effort: silently skipped when unavailable."""
    import sys, types

    if "antenv.axon_hooks" in sys.modules:
        return
    try:
        sys.path.insert(0, "/root/.axon_site")
        from trn_agent_boot.trn_boot import _ntff_profile_via_ctypes

        hook = _ntff_profile_via_ctypes("/opt/axon/libaxon_pjrt.so")
        mod = types.ModuleType("antenv.axon_hooks")
        mod._hook = hook
        mod.get_axon_ntff_profile_hook = lambda: mod._hook
        mod.set_axon_ntff_profile_hook = lambda h: setattr(mod, "_hook", h)
        import antenv

        antenv.axon_hooks = mod
        sys.modules["antenv.axon_hooks"] = mod
    except Exception:
        pass


# --------------------------------------------------------------------------
# host-side planning
# --------------------------------------------------------------------------
class Plan:
    pass


def make_plan(dst):
    deg = np.bincount(dst, minlength=N)
    assert deg.max() <= TILE_W, f"degree {deg.max()} > {TILE_W} unsupported"
    order = np.argsort(-deg, kind="stable")
    node_map = order.reshape(NPC, NC).T.copy()  # [NC, NPC]
    deg_map = deg[node_map]

    tiles = []  # (pos0, n, d)
    pos = 0
    while pos < NPC:
        d = max(int(deg_map[:, pos].max()), 1)
        n = min(TILE_W // d, NPC - pos)
        tiles.append((pos, n, d))
        pos += n

    pairs = []  # (ta, tb) tb=-1 for singleton
    i = 0
    while i < len(tiles):
        if (
            i + 1 < len(tiles)
            and tiles[i][1] == tiles[i + 1][1]
            and tiles[i][2] == tiles[i + 1][2]
        ):
            pairs.append((i, i + 1))
            i += 2
        else:
            pairs.append((i, -1))
            i += 1

    widths = [n * d for (_, n, d) in tiles]
    colstart = np.concatenate([[0], np.cumsum(widths)]).astype(np.int64)
    S = int(colstart[-1])

    outcol = []
    c = 0
    for a, b in pairs:
        outcol.append(c)
        c += tiles[a][1]

    # group adjacent 2-tile pairs of equal shape into quads (shared wide
    # PSUM tile, FD=2w ACT/DVE ops)
    groups = []  # (pair_lo, npairs in {1,2})
    i = 0
    while i < len(pairs):
        if (
            i + 1 < len(pairs)
            and pairs[i][1] >= 0
            and pairs[i + 1][1] >= 0
            and tiles[pairs[i][0]][1] == tiles[pairs[i + 1][0]][1]
            and tiles[pairs[i][0]][2] == tiles[pairs[i + 1][0]][2]
        ):
            groups.append((i, 2))
            i += 2
        else:
            groups.append((i, 1))
            i += 1

    # chunk groups into big DMA loads
    chunks = []  # (grp_lo, grp_hi, col_lo, col_hi)
    glo, clo = 0, 0
    for gi, (plo_, np_) in enumerate(groups):
        lastpair = groups[gi][0] + np_ - 1
        a, b = pairs[lastpair]
        chi = int(colstart[(b if b >= 0 else a) + 1])
        if chi - clo > CHUNK_COLS and gi > glo:
            a0 = pairs[groups[gi][0]][0]
            cmid = int(colstart[a0])
            chunks.append((glo, gi, clo, cmid))
            glo, clo = gi, cmid
    chunks.append((glo, len(groups), clo, S))
    grp_chunk = {}
    for ci, (a, b, _, _) in enumerate(chunks):
        for gi in range(a, b):
            grp_chunk[gi] = ci

    # thin rows: tile ti -> (thin tile t, row r); class runs share (n, d)
    # and never straddle a 128-row boundary
    NT = len(tiles)
    thin_t = [ti // 128 for ti in range(NT)]
    thin_r = [ti % 128 for ti in range(NT)]
    thin_runs = []  # (tile_lo, tile_hi, n, d)
    i = 0
    while i < NT:
        j = i
        while (
            j < NT
            and tiles[j][1] == tiles[i][1]
            and tiles[j][2] == tiles[i][2]
            and thin_t[j] == thin_t[i]
        ):
            j += 1
        thin_runs.append((i, j, tiles[i][1], tiles[i][2]))
        i = j
    # last tile index per thin tile (trigger for the thin chain)
    thin_last = {}
    for ti in range(NT):
        thin_last[thin_t[ti]] = ti
    NTHIN = thin_t[-1] + 1

    p = Plan()
    p.deg, p.node_map, p.deg_map = deg, node_map, deg_map
    p.tiles, p.pairs, p.colstart, p.S = tiles, pairs, colstart, S
    p.outcol, p.NCOL = np.array(outcol), c
    p.groups, p.chunks, p.grp_chunk = groups, chunks, grp_chunk
    p.thin_t, p.thin_r, p.thin_runs = thin_t, thin_r, thin_runs
    p.thin_last, p.NTHIN = thin_last, NTHIN
    return p


def make_slot_maps(plan, src, dst):
    deg = plan.deg
    eorder = np.argsort(dst, kind="stable")
    starts = np.concatenate([[0], np.cumsum(deg)]).astype(np.int64)

    slot_src = np.full((NC, plan.S), -1, np.int64)
    slot_eid = np.full((NC, plan.S), -1, np.int64)
    slot_dst = np.full((NC, plan.S), 0, np.int64)
    for ti, (pos0, n, d) in enumerate(plan.tiles):
        c0 = int(plan.colstart[ti])
        nodes = plan.node_map[:, pos0 : pos0 + n]
        degs = plan.deg_map[:, pos0 : pos0 + n]
        st = starts[nodes]
        dgrid = np.arange(d)
        eidx = st[:, :, None] + dgrid[None, None, :]
        valid = dgrid[None, None, :] < degs[:, :, None]
        eidx = np.where(valid, eidx, 0)
        eids = eorder[eidx]
        slot_eid[:, c0 : c0 + n * d] = np.where(valid, eids, -1).reshape(NC, n * d)
        slot_src[:, c0 : c0 + n * d] = np.where(valid, src[eids], -1).reshape(
            NC, n * d
        )
        slot_dst[:, c0 : c0 + n * d] = np.broadcast_to(
            nodes[:, :, None], (NC, n, d)
        ).reshape(NC, n * d)
    return slot_src, slot_eid, slot_dst


# --------------------------------------------------------------------------
# device program (shared by both layers)
# --------------------------------------------------------------------------
def build_program(plan):
    nc = bacc.Bacc("TRN2", target_bir_lowering=False, debug=False)
    S, NCOL = plan.S, plan.NCOL

    rhs_d = nc.dram_tensor("rhs", [K_RHS, S], _bf16, kind="ExternalInput")
    lmsg_d = nc.dram_tensor("lmsg", [K_RHS, DOUT], _bf16, kind="ExternalInput")
    llog_d = nc.dram_tensor("llog", [K_RHS, DOUT], _bf16, kind="ExternalInput")
    out_d = nc.dram_tensor("out", [128, NCOL], _f32, kind="ExternalOutput")
    s_d = nc.dram_tensor(
        "sthin", [128, 512 * plan.NTHIN], _f32, kind="ExternalOutput"
    )

    Exp = mybir.ActivationFunctionType.Exp

    with tile.TileContext(nc) as tc, ExitStack() as ctx:
        const = ctx.enter_context(tc.tile_pool(name="const", bufs=1))
        sb = ctx.enter_context(tc.tile_pool(name="sb", bufs=6))
        psl = ctx.enter_context(tc.tile_pool(name="psl", bufs=2, space="PSUM"))
        psm = ctx.enter_context(tc.tile_pool(name="psm", bufs=2, space="PSUM"))
        pst = ctx.enter_context(tc.tile_pool(name="pst", bufs=2, space="PSUM"))
        acc = ctx.enter_context(tc.tile_pool(name="acc", bufs=1))

        lmsg = const.tile([K_RHS, DOUT], _bf16)
        llog = const.tile([K_RHS, DOUT], _bf16)
        nc.sync.dma_start(out=lmsg[:], in_=lmsg_d[:])
        nc.sync.dma_start(out=llog[:], in_=llog_d[:])

        outacc = acc.tile([128, NCOL], _f32)
        sthin = acc.tile([128, 512 * plan.NTHIN], _f32)

        stage = {}
        thin_tiles = {}
        for gi, (plo, npair) in enumerate(plan.groups):
            ta0, tb0 = plan.pairs[plo]
            pos0, n, d = plan.tiles[ta0]
            w = n * d
            c0 = int(plan.colstart[ta0])
            oc = int(plan.outcol[plo])
            two = tb0 >= 0

            ci = plan.grp_chunk[gi]
            if ci not in stage:
                glo2, ghi2, clo, chi = plan.chunks[ci]
                st = sb.tile([K_RHS, CHUNK_COLS], _bf16, tag="stage")
                dma_eng = nc.sync if ci % 2 == 0 else nc.scalar
                dma_eng.dma_start(out=st[:, : chi - clo], in_=rhs_d[:, clo:chi])
                stage = {ci: (st, clo)}
            st, clo = stage[ci]
            s0 = c0 - clo

            # psum tiles: plog wide [128, npair, 512]; pmsg per pair
            plog = psl.tile([128, 2, TILE_W], _f32, tag="plog")

            np_ = 128 if two else 64
            thin_done = []
            for q in range(npair):
                ta, tb = plan.pairs[plo + q]
                cq = int(plan.colstart[ta]) - clo
                nc.tensor.matmul(
                    out=plog[0:64, q, :w],
                    lhsT=llog[:],
                    rhs=st[:, cq : cq + w],
                    start=True,
                    stop=True,
                )
                if two:
                    nc.tensor.matmul(
                        out=plog[64:128, q, :w],
                        lhsT=llog[:],
                        rhs=st[:, cq + w : cq + 2 * w],
                        start=True,
                        stop=True,
                    )
                # thin logit rows (one per tile) for the softmax sums
                for side, ti in enumerate((ta, tb)):
                    if ti < 0:
                        continue
                    t = plan.thin_t[ti]
                    r = plan.thin_r[ti]
                    if t not in thin_tiles:
                        thin_tiles[t] = pst.tile(
                            [128, TILE_W], _f32, tag="thin"
                        )
                    cqt = cq + side * w
                    nc.tensor.matmul(
                        out=thin_tiles[t][r : r + 1, :w],
                        lhsT=llog[:, 0:1],
                        rhs=st[:, cqt : cqt + w],
                        start=True,
                        stop=True,
                    )
                    if ti == plan.thin_last[t]:
                        thin_done.append(t)

            pv = plog[0:np_, 0:npair, :w]
            pt = sb.tile([128, 2, TILE_W], _bf16, tag="p")
            pt2 = sb.tile([128, 2, TILE_W], _bf16, tag="p2")
            nc.scalar.activation(out=pt[0:np_, 0:npair, :w], in_=pv, func=Exp)
            nc.scalar.activation(
                out=pt2[0:np_, 0:npair, :w], in_=pv, func=Exp, scale=ATT_SLOPE
            )
            nc.vector.tensor_max(
                out=pt[0:np_, 0:npair, :w],
                in0=pt[0:np_, 0:npair, :w],
                in1=pt2[0:np_, 0:npair, :w],
            )
            for q in range(npair):
                ta, tb = plan.pairs[plo + q]
                cq = int(plan.colstart[ta]) - clo
                pmsg = psm.tile([128, TILE_W], _f32, tag="pmsg")
                nc.tensor.matmul(
                    out=pmsg[0:64, :w],
                    lhsT=lmsg[:],
                    rhs=st[:, cq : cq + w],
                    start=True,
                    stop=True,
                )
                if two:
                    nc.tensor.matmul(
                        out=pmsg[64:128, :w],
                        lhsT=lmsg[:],
                        rhs=st[:, cq + w : cq + 2 * w],
                        start=True,
                        stop=True,
                    )
                mp = sb.tile([128, TILE_W], _bf16, tag="mp")
                nc.vector.tensor_mul(
                    out=mp[0:np_, :w],
                    in0=pmsg[0:np_, :w],
                    in1=pt[0:np_, q, :w],
                )
                nc.vector.tensor_reduce(
                    out=outacc[0:np_, oc + q * n : oc + (q + 1) * n],
                    in_=mp[0:np_, :w].rearrange("p (n d) -> p n d", d=d),
                    axis=mybir.AxisListType.X,
                    op=mybir.AluOpType.max,
                )
            if not two:
                nc.vector.memset(outacc[64:128, oc : oc + n], 0.0)

            # drain any completed thin tiles: exp both slopes, max, then
            # per-class-run segmented sums into sthin
            for t in thin_done:
                tps = thin_tiles.pop(t)
                ptt = sb.tile([128, TILE_W], _bf16, tag="ptt")
                ptt2 = sb.tile([128, TILE_W], _bf16, tag="ptt2")
                nc.scalar.activation(out=ptt[:], in_=tps[:], func=Exp)
                nc.scalar.activation(
                    out=ptt2[:], in_=tps[:], func=Exp, scale=ATT_SLOPE
                )
                nc.vector.tensor_max(out=ptt[:], in0=ptt[:], in1=ptt2[:])
                for lo, hi, rn, rd in plan.thin_runs:
                    if plan.thin_t[lo] != t:
                        continue
                    r0, r1 = plan.thin_r[lo], plan.thin_r[hi - 1] + 1
                    nc.vector.tensor_reduce(
                        out=sthin[r0:r1, 512 * t : 512 * t + rn],
                        in_=ptt[r0:r1, 0 : rn * rd].rearrange(
                            "p (n d) -> p n d", d=rd
                        ),
                        axis=mybir.AxisListType.X,
                        op=mybir.AluOpType.add,
                    )

        # ---- finalize: division/bias/activation happen on the host
        nc.sync.dma_start(out=out_d[:], in_=outacc[:])
        nc.sync.dma_start(out=s_d[:], in_=sthin[:])

    nc.compile()
    return nc


# --------------------------------------------------------------------------
# launches + assembly
# --------------------------------------------------------------------------
def make_lhs(W, We, a_s, a_e):
    lmsg = np.zeros((K_RHS, DOUT), np.float32)
    lmsg[:DIN] = W
    lmsg[ROW_EA : ROW_EA + DE] = We
    lmsg[ROW_PAD, :] = BIG_NEG
    llog = np.zeros((K_RHS, DOUT), np.float32)
    llog[:DIN] = (W @ a_s)[:, None]
    llog[ROW_EA : ROW_EA + DE] = (We @ a_e)[:, None]
    llog[ROW_AD, :] = 1.0
    llog[ROW_PAD, :] = PAD_LOGIT
    return lmsg, llog


def assemble(plan, outs, sthins, b):
    """Division by the softmax sum, empty fixup, +b and leaky-relu happen
    here (host, O(N*DOUT) elementwise); all O(E) work is on-device."""
    full = np.zeros((N, DOUT), np.float32)
    for pi, (ta, tb) in enumerate(plan.pairs):
        pos0, n, d = plan.tiles[ta]
        oc = int(plan.outcol[pi])
        for side, ti in enumerate((ta, tb)):
            if ti < 0:
                continue
            pos0t = plan.tiles[ti][0]
            r = plan.thin_r[ti]
            cb = 512 * plan.thin_t[ti]
            s = sthins[:, r, cb : cb + n]  # [NC, n]
            v = outs[:, 64 * side : 64 * side + 64, oc : oc + n] / s[:, None, :]
            v = np.where(v >= EMPTY_THR, v, 0.0)
            for c in range(NC):
                nodes = plan.node_map[c, pos0t : pos0t + n]
                full[nodes] = v[c].T
    full += b
    return np.where(full >= 0, full, ACT_SLOPE * full)


def kernel(
    X,
    edge_index,
    edge_attr,
    W1,
    We1,
    as1,
    ad1,
    ae1,
    b1,
    W2,
    We2,
    as2,
    ad2,
    ae2,
    b2,
):
    trace = os.environ.get("GAT_TRACE") == "1"
    if trace:
        _install_ntff_shim()
    LAST_EXEC_NS.clear()
    X = np.asarray(X, np.float32)
    edge_attr = np.asarray(edge_attr, np.float32)
    src = np.asarray(edge_index[0], np.int64)
    dst = np.asarray(edge_index[1], np.int64)
    W1, We1, as1, ad1, ae1, b1 = [
        np.asarray(a, np.float32) for a in (W1, We1, as1, ad1, ae1, b1)
    ]
    W2, We2, as2, ad2, ae2, b2 = [
        np.asarray(a, np.float32) for a in (W2, We2, as2, ad2, ae2, b2)
    ]

    plan = make_plan(dst)
    slot_src, slot_eid, slot_dst = make_slot_maps(plan, src, dst)

    # edge-attr part of the stream (rows 64:80) + pad flag, both layers
    valid_e = slot_eid >= 0
    ea = edge_attr[np.where(valid_e, slot_eid, 0)]
    ea[~valid_e] = 0.0
    ea_part = ea.transpose(0, 2, 1)  # [NC, DE, S]
    pad_part = (~valid_e).astype(np.float32)  # [NC, S]
    del ea

    nc_prog = build_program(plan)

    valid_s = slot_src >= 0

    def layer(node_feat, W, We, a_s, a_e, a_d, b):
        rhs = np.zeros((NC, K_RHS, plan.S), np.float32)
        xs = node_feat[np.where(valid_s, slot_src, 0)]
        xs[~valid_s] = 0.0
        rhs[:, :DIN, :] = xs.transpose(0, 2, 1)
        rhs[:, ROW_EA : ROW_EA + DE, :] = ea_part
        ad = node_feat @ (W @ a_d)  # [N]
        rhs[:, ROW_AD, :] = np.where(valid_e, ad[slot_dst], 0.0)
        rhs[:, ROW_PAD, :] = pad_part
        lmsg, llog = make_lhs(W, We, a_s, a_e)
        rhs16 = _bf(rhs)
        in_maps = [
            {
                "rhs": rhs16[c],
                "lmsg": _bf(lmsg),
                "llog": _bf(llog),
            }
            for c in range(NC)
        ]
        res = run_bass_kernel_spmd(
            nc_prog, in_maps, core_ids=list(range(NC)), trace=trace
        )
        if trace and res.exec_time_ns:
            LAST_EXEC_NS.append(res.exec_time_ns)
        if os.environ.get("GAT_DUMP_TRACE") == "1" and res.instructions_and_trace:
            import pickle

            insts = res.instructions_and_trace[0]

            def _s(v):
                return v() if callable(v) else v

            rows = [
                (
                    str(i.engine),
                    str(_s(i.op_name)),
                    i.timestamp,
                    i.duration,
                    i.evt_wait_time,
                    str(_s(i.name)),
                )
                for i in insts
            ]
            with open(f"/tmp/gat_insts_{len(LAST_EXEC_NS)}.pkl", "wb") as f:
                pickle.dump(rows, f)
        outs = np.stack([res.results[c]["out"] for c in range(NC)])
        sthins = np.stack([res.results[c]["sthin"] for c in range(NC)])
        return assemble(plan, outs, sthins, b)

    c1 = layer(X, W1, We1, as1, ae1, ad1, b1)
    c2 = layer(c1, W2, We2, as2, ae2, ad2, b2)
    return c2


# revision 14
# speedup vs baseline: 1.0095x; 1.0095x over previous
"""2-layer GAT (edge features, softmax attention over dst, max aggregation)
on 8 TRN2 NeuronCores — dst-sharded, edge-slot streaming formulation.

Original staged baseline (HW exec ~1.03ms). Kept as fallback.
"""

import os
import numpy as np
import ml_dtypes
from contextlib import ExitStack

import concourse.bacc as bacc
import concourse.bass as bass
import concourse.mybir as mybir
import concourse.tile as tile
from concourse.bass_utils import run_bass_kernel_spmd

N = 50000
E = 1600000
DIN = 64
DOUT = 64
DE = 16
NC = 8
NPC = N // NC
ATT_SLOPE = 0.2
ACT_SLOPE = 0.01
PAD_LOGIT = -150.0
BIG_NEG = -1.0e30
EMPTY_THR = -1.0e6
K_RHS = DIN + DE + 1  # 81: x(0:64), ea(64:80), pad(80)
ROW_EA = DIN
ROW_PAD = DIN + DE
CHUNK_COLS = 8192
TILE_W = 512

LAST_EXEC_NS = []

_bf16 = mybir.dt.bfloat16
_f32 = mybir.dt.float32


def _bf(a):
    return np.asarray(a, np.float32).astype(ml_dtypes.bfloat16)


def _install_ntff_shim():
    import sys, types

    if "antenv.axon_hooks" in sys.modules:
        return
    try:
        sys.path.insert(0, "/root/.axon_site")
        from trn_agent_boot.trn_boot import _ntff_profile_via_ctypes

        hook = _ntff_profile_via_ctypes("/opt/axon/libaxon_pjrt.so")
        mod = types.ModuleType("antenv.axon_hooks")
        mod._hook = hook
        mod.get_axon_ntff_profile_hook = lambda: mod._hook
        mod.set_axon_ntff_profile_hook = lambda h: setattr(mod, "_hook", h)
        import antenv

        antenv.axon_hooks = mod
        sys.modules["antenv.axon_hooks"] = mod
    except Exception:
        pass


class Plan:
    pass


def make_plan(dst):
    deg = np.bincount(dst, minlength=N)
    assert deg.max() <= TILE_W, f"degree {deg.max()} > {TILE_W} unsupported"
    order = np.argsort(-deg, kind="stable")
    node_map = order.reshape(NPC, NC).T.copy()  # [NC, NPC]
    deg_map = deg[node_map]

    tiles = []  # (pos0, n, d)
    pos = 0
    while pos < NPC:
        d = max(int(deg_map[:, pos].max()), 1)
        n = min(TILE_W // d, NPC - pos)
        tiles.append((pos, n, d))
        pos += n

    pairs = []  # (ta, tb) tb=-1 for singleton
    i = 0
    while i < len(tiles):
        if (
            i + 1 < len(tiles)
            and tiles[i][1] == tiles[i + 1][1]
            and tiles[i][2] == tiles[i + 1][2]
        ):
            pairs.append((i, i + 1))
            i += 2
        else:
            pairs.append((i, -1))
            i += 1

    widths = [n * d for (_, n, d) in tiles]
    colstart = np.concatenate([[0], np.cumsum(widths)]).astype(np.int64)
    S = int(colstart[-1])

    outcol = []
    c = 0
    for a, b in pairs:
        outcol.append(c)
        c += tiles[a][1]

    classes = []
    i = 0
    while i < len(tiles):
        j = i
        while j < len(tiles) and tiles[j][2] == tiles[i][2]:
            j += 1
        classes.append((i, j, tiles[i][2]))
        i = j

    chunks = []
    plo, clo = 0, 0
    for pi, (a, b) in enumerate(pairs):
        chi = int(colstart[(b if b >= 0 else a) + 1])
        if chi - clo > CHUNK_COLS and pi > plo:
            cmid = int(colstart[pairs[pi][0]])
            chunks.append((plo, pi, clo, cmid))
            plo, clo = pi, cmid
    chunks.append((plo, len(pairs), clo, S))
    pair_chunk = {}
    for ci, (a, b, _, _) in enumerate(chunks):
        for pi in range(a, b):
            pair_chunk[pi] = ci

    p = Plan()
    p.deg, p.node_map, p.deg_map = deg, node_map, deg_map
    p.tiles, p.pairs, p.colstart, p.S = tiles, pairs, colstart, S
    p.outcol, p.NCOL, p.classes = np.array(outcol), c, classes
    p.chunks, p.pair_chunk = chunks, pair_chunk
    return p


def make_slot_maps(plan, src, dst):
    deg = plan.deg
    eorder = np.argsort(dst, kind="stable")
    starts = np.concatenate([[0], np.cumsum(deg)]).astype(np.int64)

    slot_src = np.full((NC, plan.S), -1, np.int64)
    slot_eid = np.full((NC, plan.S), -1, np.int64)
    for ti, (pos0, n, d) in enumerate(plan.tiles):
        c0 = int(plan.colstart[ti])
        nodes = plan.node_map[:, pos0 : pos0 + n]
        degs = plan.deg_map[:, pos0 : pos0 + n]
        st = starts[nodes]
        dgrid = np.arange(d)
        eidx = st[:, :, None] + dgrid[None, None, :]
        valid = dgrid[None, None, :] < degs[:, :, None]
        eidx = np.where(valid, eidx, 0)
        eids = eorder[eidx]
        slot_eid[:, c0 : c0 + n * d] = np.where(valid, eids, -1).reshape(NC, n * d)
        slot_src[:, c0 : c0 + n * d] = np.where(valid, src[eids], -1).reshape(
            NC, n * d
        )
    return slot_src, slot_eid


def build_program(plan):
    nc = bacc.Bacc("TRN2", target_bir_lowering=False, debug=False)
    S, NCOL = plan.S, plan.NCOL

    rhs_d = nc.dram_tensor("rhs", [K_RHS, S], _bf16, kind="ExternalInput")
    xperm_d = nc.dram_tensor("xperm", [DIN, NPC], _bf16, kind="ExternalInput")
    lmsg_d = nc.dram_tensor("lmsg", [K_RHS, DOUT], _bf16, kind="ExternalInput")
    llog_d = nc.dram_tensor("llog", [K_RHS, DOUT], _bf16, kind="ExternalInput")
    wad_d = nc.dram_tensor("wad", [DIN, 1], _bf16, kind="ExternalInput")
    bvec_d = nc.dram_tensor("bvec", [128, 1], _f32, kind="ExternalInput")
    ones_d = nc.dram_tensor("ones", [1, DOUT], _bf16, kind="ExternalInput")
    out_d = nc.dram_tensor("out", [128, NCOL], _f32, kind="ExternalOutput")

    with tile.TileContext(nc) as tc, ExitStack() as ctx:
        const = ctx.enter_context(tc.tile_pool(name="const", bufs=1))
        sb = ctx.enter_context(tc.tile_pool(name="sb", bufs=6))
        ps = ctx.enter_context(tc.tile_pool(name="ps", bufs=3, space="PSUM"))
        acc = ctx.enter_context(tc.tile_pool(name="acc", bufs=1))
        psa = ctx.enter_context(tc.tile_pool(name="psa", bufs=2, space="PSUM"))

        lmsg = const.tile([K_RHS, DOUT], _bf16)
        llog = const.tile([K_RHS, DOUT], _bf16)
        wad = const.tile([DIN, 1], _bf16)
        bvec = const.tile([128, 1], _f32)
        ones = const.tile([1, DOUT], _bf16)
        nc.sync.dma_start(out=ones[:], in_=ones_d[:])
        nc.sync.dma_start(out=lmsg[:], in_=lmsg_d[:])
        nc.sync.dma_start(out=llog[:], in_=llog_d[:])
        nc.sync.dma_start(out=wad[:], in_=wad_d[:])
        nc.sync.dma_start(out=bvec[:], in_=bvec_d[:])

        xperm = const.tile([DIN, NPC], _bf16)
        nc.sync.dma_start(out=xperm[:], in_=xperm_d[:])
        ad_sb = const.tile([1, NPC], _bf16)
        for j0 in range(0, NPC, TILE_W):
            w = min(TILE_W, NPC - j0)
            ap_ = psa.tile([1, TILE_W], _f32, tag="adps")
            nc.tensor.matmul(
                out=ap_[:, :w],
                lhsT=wad[:],
                rhs=xperm[:, j0 : j0 + w],
                start=True,
                stop=True,
            )
            nc.vector.tensor_copy(out=ad_sb[:, j0 : j0 + w], in_=ap_[:, :w])

        outacc = acc.tile([128, NCOL], _f32)
        sacc = acc.tile([128, NCOL], _f32)

        stage = {}
        for pi, (ta, tb) in enumerate(plan.pairs):
            pos0, n, d = plan.tiles[ta]
            w = n * d
            c0 = int(plan.colstart[ta])
            oc = int(plan.outcol[pi])
            two = tb >= 0
            wtot = 2 * w if two else w

            ci = plan.pair_chunk[pi]
            if ci not in stage:
                plo, phi, clo, chi = plan.chunks[ci]
                st = sb.tile([K_RHS, CHUNK_COLS], _bf16, tag="stage")
                dma_eng = nc.sync if ci % 2 == 0 else nc.scalar
                dma_eng.dma_start(out=st[:, : chi - clo], in_=rhs_d[:, clo:chi])
                stage = {ci: (st, clo)}
            st, clo = stage[ci]
            s0 = c0 - clo
            rt = st[:, s0 : s0 + wtot]

            pmsg = ps.tile([128, TILE_W], _f32, tag="pmsg")
            plog = ps.tile([128, TILE_W], _f32, tag="plog")
            pos0b = plan.tiles[tb][0] if two else 0
            nc.tensor.matmul(
                out=pmsg[0:64, :w], lhsT=lmsg[:], rhs=rt[:, :w], start=True, stop=True
            )
            if two:
                nc.tensor.matmul(
                    out=pmsg[64:128, :w],
                    lhsT=lmsg[:],
                    rhs=rt[:, w : 2 * w],
                    start=True,
                    stop=True,
                )
            nc.tensor.matmul(
                out=plog[0:64, :w], lhsT=llog[:], rhs=rt[:, :w], start=True, stop=False
            )
            if two:
                nc.tensor.matmul(
                    out=plog[64:128, :w],
                    lhsT=llog[:],
                    rhs=rt[:, w : 2 * w],
                    start=True,
                    stop=False,
                )
            nc.tensor.matmul(
                out=plog[0:64, :w],
                lhsT=ones[:],
                rhs=ad_sb[:, pos0 : pos0 + n].unsqueeze(2).broadcast_to([1, n, d]),
                start=False,
                stop=True,
            )
            if two:
                nc.tensor.matmul(
                    out=plog[64:128, :w],
                    lhsT=ones[:],
                    rhs=ad_sb[:, pos0b : pos0b + n]
                    .unsqueeze(2)
                    .broadcast_to([1, n, d]),
                    start=False,
                    stop=True,
                )
            np_ = 128 if two else 64

            pt = sb.tile([128, TILE_W], _bf16, tag="p")
            pt2 = sb.tile([128, TILE_W], _bf16, tag="p2")
            nc.scalar.activation(
                out=pt[:np_, :w],
                in_=plog[:np_, :w],
                func=mybir.ActivationFunctionType.Exp,
            )
            nc.scalar.activation(
                out=pt2[:np_, :w],
                in_=plog[:np_, :w],
                func=mybir.ActivationFunctionType.Exp,
                scale=ATT_SLOPE,
            )
            nc.vector.tensor_max(
                out=pt[:np_, :w], in0=pt[:np_, :w], in1=pt2[:np_, :w]
            )
            mp = sb.tile([128, TILE_W], _bf16, tag="mp")
            nc.vector.tensor_mul(out=mp[:np_, :w], in0=pmsg[:np_, :w], in1=pt[:np_, :w])
            nc.vector.tensor_reduce(
                out=outacc[:np_, oc : oc + n],
                in_=mp[:np_, :w].rearrange("p (n d) -> p n d", d=d),
                axis=mybir.AxisListType.X,
                op=mybir.AluOpType.max,
            )
            nc.vector.tensor_reduce(
                out=sacc[:np_, oc : oc + n],
                in_=pt[:np_, :w].rearrange("p (n d) -> p n d", d=d),
                axis=mybir.AxisListType.X,
                op=mybir.AluOpType.add,
            )
            if not two:
                nc.vector.memset(outacc[64:128, oc : oc + n], 0.0)
                nc.vector.memset(sacc[64:128, oc : oc + n], 1.0)

        rs = acc.tile([128, NCOL], _f32)
        nc.vector.reciprocal(out=rs[:], in_=sacc[:])
        nc.vector.tensor_mul(out=outacc[:], in0=outacc[:], in1=rs[:])
        mask = acc.tile([128, NCOL], _f32)
        nc.vector.tensor_scalar(
            out=mask[:],
            in0=outacc[:],
            scalar1=float(EMPTY_THR),
            scalar2=None,
            op0=mybir.AluOpType.is_ge,
        )
        nc.vector.tensor_mul(out=outacc[:], in0=outacc[:], in1=mask[:])
        nc.vector.tensor_scalar(
            out=outacc[:],
            in0=outacc[:],
            scalar1=bvec[:],
            scalar2=None,
            op0=mybir.AluOpType.add,
        )
        nc.vector.scalar_tensor_tensor(
            out=outacc[:],
            in0=outacc[:],
            scalar=ACT_SLOPE,
            in1=outacc[:],
            op0=mybir.AluOpType.mult,
            op1=mybir.AluOpType.max,
        )
        nc.sync.dma_start(out=out_d[:], in_=outacc[:])

    nc.compile()
    return nc


def make_lhs(W, We, a_s, a_e):
    lmsg = np.zeros((K_RHS, DOUT), np.float32)
    lmsg[:DIN] = W
    lmsg[ROW_EA : ROW_EA + DE] = We
    lmsg[ROW_PAD, :] = BIG_NEG
    llog = np.zeros((K_RHS, DOUT), np.float32)
    llog[:DIN] = (W @ a_s)[:, None]
    llog[ROW_EA : ROW_EA + DE] = (We @ a_e)[:, None]
    llog[ROW_PAD, :] = PAD_LOGIT
    return lmsg, llog


def assemble(plan, outs):
    full = np.zeros((N, DOUT), np.float32)
    for pi, (ta, tb) in enumerate(plan.pairs):
        pos0, n, d = plan.tiles[ta]
        oc = int(plan.outcol[pi])
        for c in range(NC):
            nodes = plan.node_map[c, pos0 : pos0 + n]
            full[nodes] = outs[c, 0:64, oc : oc + n].T
            if tb >= 0:
                pos0b, nb, _ = plan.tiles[tb]
                nodesb = plan.node_map[c, pos0b : pos0b + nb]
                full[nodesb] = outs[c, 64:128, oc : oc + n].T
    return full


def kernel(
    X,
    edge_index,
    edge_attr,
    W1,
    We1,
    as1,
    ad1,
    ae1,
    b1,
    W2,
    We2,
    as2,
    ad2,
    ae2,
    b2,
):
    trace = os.environ.get("GAT_TRACE") == "1"
    if trace:
        _install_ntff_shim()
    LAST_EXEC_NS.clear()
    X = np.asarray(X, np.float32)
    edge_attr = np.asarray(edge_attr, np.float32)
    src = np.asarray(edge_index[0], np.int64)
    dst = np.asarray(edge_index[1], np.int64)
    W1, We1, as1, ad1, ae1, b1 = [
        np.asarray(a, np.float32) for a in (W1, We1, as1, ad1, ae1, b1)
    ]
    W2, We2, as2, ad2, ae2, b2 = [
        np.asarray(a, np.float32) for a in (W2, We2, as2, ad2, ae2, b2)
    ]

    plan = make_plan(dst)
    slot_src, slot_eid = make_slot_maps(plan, src, dst)

    valid_e = slot_eid >= 0
    ea = edge_attr[np.where(valid_e, slot_eid, 0)]
    ea[~valid_e] = 0.0
    ea_part = np.zeros((NC, DE + 1, plan.S), np.float32)
    ea_part[:, :DE, :] = ea.transpose(0, 2, 1)
    ea_part[:, DE, :] = (~valid_e).astype(np.float32)
    del ea

    nc_prog = build_program(plan)

    valid_s = slot_src >= 0

    def layer(node_feat, W, We, a_s, a_e, a_d, b):
        rhs = np.zeros((NC, K_RHS, plan.S), np.float32)
        xs = node_feat[np.where(valid_s, slot_src, 0)]
        xs[~valid_s] = 0.0
        rhs[:, :DIN, :] = xs.transpose(0, 2, 1)
        rhs[:, ROW_EA : ROW_EA + DE + 1, :] = ea_part
        xperm = node_feat[plan.node_map].transpose(0, 2, 1)
        lmsg, llog = make_lhs(W, We, a_s, a_e)
        wad = (W @ a_d)[:, None]
        bvec = np.concatenate([b, b]).reshape(128, 1).astype(np.float32)
        rhs16, xperm16 = _bf(rhs), np.ascontiguousarray(_bf(xperm))
        in_maps = [
            {
                "rhs": rhs16[c],
                "xperm": xperm16[c],
                "lmsg": _bf(lmsg),
                "llog": _bf(llog),
                "wad": _bf(wad),
                "bvec": bvec,
                "ones": np.ones((1, DOUT), ml_dtypes.bfloat16),
            }
            for c in range(NC)
        ]
        res = run_bass_kernel_spmd(
            nc_prog, in_maps, core_ids=list(range(NC)), trace=trace
        )
        if trace and res.exec_time_ns:
            LAST_EXEC_NS.append(res.exec_time_ns)
        outs = np.stack([res.results[c]["out"] for c in range(NC)])
        return assemble(plan, outs)

    c1 = layer(X, W1, We1, as1, ae1, ad1, b1)
    c2 = layer(c1, W2, We2, as2, ae2, ad2, b2)
    return c2


# revision 15
# speedup vs baseline: 1.4330x; 1.4196x over previous
"""2-layer GAT (edge features, softmax attention over dst, max aggregation)
on 8 TRN2 NeuronCores — dst-sharded, edge-slot streaming formulation.

Original staged baseline (HW exec ~1.03ms). Kept as fallback.
"""

import os
import numpy as np
import ml_dtypes
from contextlib import ExitStack

import concourse.bacc as bacc
import concourse.bass as bass
import concourse.mybir as mybir
import concourse.tile as tile
from concourse.bass_utils import run_bass_kernel_spmd

N = 50000
E = 1600000
DIN = 64
DOUT = 64
DE = 16
NC = 8
NPC = N // NC
ATT_SLOPE = 0.2
ACT_SLOPE = 0.01
PAD_LOGIT = -150.0
BIG_NEG = -1.0e30
EMPTY_THR = -1.0e6
K_RHS = DIN + DE + 1  # 81: x(0:64), ea(64:80), pad(80)
ROW_EA = DIN
ROW_PAD = DIN + DE
CHUNK_COLS = 8192
TILE_W = 512

LAST_EXEC_NS = []

_bf16 = mybir.dt.bfloat16
_f32 = mybir.dt.float32


def _bf(a):
    return np.asarray(a, np.float32).astype(ml_dtypes.bfloat16)


def _install_ntff_shim():
    import sys, types

    if "antenv.axon_hooks" in sys.modules:
        return
    try:
        sys.path.insert(0, "/root/.axon_site")
        from trn_agent_boot.trn_boot import _ntff_profile_via_ctypes

        hook = _ntff_profile_via_ctypes("/opt/axon/libaxon_pjrt.so")
        mod = types.ModuleType("antenv.axon_hooks")
        mod._hook = hook
        mod.get_axon_ntff_profile_hook = lambda: mod._hook
        mod.set_axon_ntff_profile_hook = lambda h: setattr(mod, "_hook", h)
        import antenv

        antenv.axon_hooks = mod
        sys.modules["antenv.axon_hooks"] = mod
    except Exception:
        pass


class Plan:
    pass


def make_plan(dst):
    deg = np.bincount(dst, minlength=N)
    assert deg.max() <= TILE_W, f"degree {deg.max()} > {TILE_W} unsupported"
    order = np.argsort(-deg, kind="stable")
    node_map = order.reshape(NPC, NC).T.copy()  # [NC, NPC]
    deg_map = deg[node_map]

    tiles = []  # (pos0, n, d)
    pos = 0
    while pos < NPC:
        d = max(int(deg_map[:, pos].max()), 1)
        n = min(TILE_W // d, NPC - pos)
        tiles.append((pos, n, d))
        pos += n

    pairs = []  # (ta, tb) tb=-1 for singleton
    i = 0
    while i < len(tiles):
        if (
            i + 1 < len(tiles)
            and tiles[i][1] == tiles[i + 1][1]
            and tiles[i][2] == tiles[i + 1][2]
        ):
            pairs.append((i, i + 1))
            i += 2
        else:
            pairs.append((i, -1))
            i += 1

    widths = [n * d for (_, n, d) in tiles]
    colstart = np.concatenate([[0], np.cumsum(widths)]).astype(np.int64)
    S = int(colstart[-1])

    outcol = []
    c = 0
    for a, b in pairs:
        outcol.append(c)
        c += tiles[a][1]

    classes = []
    i = 0
    while i < len(tiles):
        j = i
        while j < len(tiles) and tiles[j][2] == tiles[i][2]:
            j += 1
        classes.append((i, j, tiles[i][2]))
        i = j

    chunks = []
    plo, clo = 0, 0
    for pi, (a, b) in enumerate(pairs):
        chi = int(colstart[(b if b >= 0 else a) + 1])
        if chi - clo > CHUNK_COLS and pi > plo:
            cmid = int(colstart[pairs[pi][0]])
            chunks.append((plo, pi, clo, cmid))
            plo, clo = pi, cmid
    chunks.append((plo, len(pairs), clo, S))
    pair_chunk = {}
    for ci, (a, b, _, _) in enumerate(chunks):
        for pi in range(a, b):
            pair_chunk[pi] = ci

    p = Plan()
    p.deg, p.node_map, p.deg_map = deg, node_map, deg_map
    p.tiles, p.pairs, p.colstart, p.S = tiles, pairs, colstart, S
    p.outcol, p.NCOL, p.classes = np.array(outcol), c, classes
    p.chunks, p.pair_chunk = chunks, pair_chunk
    return p


def make_slot_maps(plan, src, dst):
    deg = plan.deg
    eorder = np.argsort(dst, kind="stable")
    starts = np.concatenate([[0], np.cumsum(deg)]).astype(np.int64)

    slot_src = np.full((NC, plan.S), -1, np.int64)
    slot_eid = np.full((NC, plan.S), -1, np.int64)
    for ti, (pos0, n, d) in enumerate(plan.tiles):
        c0 = int(plan.colstart[ti])
        nodes = plan.node_map[:, pos0 : pos0 + n]
        degs = plan.deg_map[:, pos0 : pos0 + n]
        st = starts[nodes]
        dgrid = np.arange(d)
        eidx = st[:, :, None] + dgrid[None, None, :]
        valid = dgrid[None, None, :] < degs[:, :, None]
        eidx = np.where(valid, eidx, 0)
        eids = eorder[eidx]
        slot_eid[:, c0 : c0 + n * d] = np.where(valid, eids, -1).reshape(NC, n * d)
        slot_src[:, c0 : c0 + n * d] = np.where(valid, src[eids], -1).reshape(
            NC, n * d
        )
    return slot_src, slot_eid


def build_program(plan):
    nc = bacc.Bacc("TRN2", target_bir_lowering=False, debug=False)
    S, NCOL = plan.S, plan.NCOL

    rhs_d = nc.dram_tensor("rhs", [K_RHS, S], _bf16, kind="ExternalInput")
    xperm_d = nc.dram_tensor("xperm", [DIN, NPC], _bf16, kind="ExternalInput")
    lmsg_d = nc.dram_tensor("lmsg", [K_RHS, DOUT], _bf16, kind="ExternalInput")
    llog_d = nc.dram_tensor("llog", [K_RHS, DOUT], _bf16, kind="ExternalInput")
    wad_d = nc.dram_tensor("wad", [DIN, 1], _bf16, kind="ExternalInput")
    bvec_d = nc.dram_tensor("bvec", [128, 1], _f32, kind="ExternalInput")
    ones_d = nc.dram_tensor("ones", [1, DOUT], _bf16, kind="ExternalInput")
    out_d = nc.dram_tensor("out", [128, NCOL], _f32, kind="ExternalOutput")

    with tile.TileContext(nc) as tc, ExitStack() as ctx:
        const = ctx.enter_context(tc.tile_pool(name="const", bufs=1))
        sb = ctx.enter_context(tc.tile_pool(name="sb", bufs=6))
        ps = ctx.enter_context(tc.tile_pool(name="ps", bufs=3, space="PSUM"))
        acc = ctx.enter_context(tc.tile_pool(name="acc", bufs=1))
        psa = ctx.enter_context(tc.tile_pool(name="psa", bufs=2, space="PSUM"))

        lmsg = const.tile([K_RHS, DOUT], _bf16)
        llog = const.tile([K_RHS, DOUT], _bf16)
        wad = const.tile([DIN, 1], _bf16)
        bvec = const.tile([128, 1], _f32)
        ones = const.tile([1, DOUT], _bf16)
        nc.sync.dma_start(out=ones[:], in_=ones_d[:])
        nc.sync.dma_start(out=lmsg[:], in_=lmsg_d[:])
        nc.sync.dma_start(out=llog[:], in_=llog_d[:])
        nc.sync.dma_start(out=wad[:], in_=wad_d[:])
        nc.sync.dma_start(out=bvec[:], in_=bvec_d[:])

        xperm = const.tile([DIN, NPC], _bf16)
        nc.sync.dma_start(out=xperm[:], in_=xperm_d[:])
        ad_sb = const.tile([1, NPC], _bf16)
        for j0 in range(0, NPC, TILE_W):
            w = min(TILE_W, NPC - j0)
            ap_ = psa.tile([1, TILE_W], _f32, tag="adps")
            nc.tensor.matmul(
                out=ap_[:, :w],
                lhsT=wad[:],
                rhs=xperm[:, j0 : j0 + w],
                start=True,
                stop=True,
            )
            nc.vector.tensor_copy(out=ad_sb[:, j0 : j0 + w], in_=ap_[:, :w])

        outacc = acc.tile([128, NCOL], _f32)
        sacc = acc.tile([128, NCOL], _f32)

        stage = {}
        for pi, (ta, tb) in enumerate(plan.pairs):
            pos0, n, d = plan.tiles[ta]
            w = n * d
            c0 = int(plan.colstart[ta])
            oc = int(plan.outcol[pi])
            two = tb >= 0
            wtot = 2 * w if two else w

            ci = plan.pair_chunk[pi]
            if ci not in stage:
                plo, phi, clo, chi = plan.chunks[ci]
                st = sb.tile([K_RHS, CHUNK_COLS], _bf16, tag="stage")
                dma_eng = nc.sync if ci % 2 == 0 else nc.scalar
                dma_eng.dma_start(out=st[:, : chi - clo], in_=rhs_d[:, clo:chi])
                stage = {ci: (st, clo)}
            st, clo = stage[ci]
            s0 = c0 - clo
            rt = st[:, s0 : s0 + wtot]

            pmsg = ps.tile([128, TILE_W], _f32, tag="pmsg")
            plog = ps.tile([128, TILE_W], _f32, tag="plog")
            pos0b = plan.tiles[tb][0] if two else 0
            nc.tensor.matmul(
                out=pmsg[0:64, :w], lhsT=lmsg[:], rhs=rt[:, :w], start=True, stop=True
            )
            if two:
                nc.tensor.matmul(
                    out=pmsg[64:128, :w],
                    lhsT=lmsg[:],
                    rhs=rt[:, w : 2 * w],
                    start=True,
                    stop=True,
                )
            nc.tensor.matmul(
                out=plog[0:64, :w], lhsT=llog[:], rhs=rt[:, :w], start=True, stop=False
            )
            if two:
                nc.tensor.matmul(
                    out=plog[64:128, :w],
                    lhsT=llog[:],
                    rhs=rt[:, w : 2 * w],
                    start=True,
                    stop=False,
                )
            nc.tensor.matmul(
                out=plog[0:64, :w],
                lhsT=ones[:],
                rhs=ad_sb[:, pos0 : pos0 + n].unsqueeze(2).broadcast_to([1, n, d]),
                start=False,
                stop=True,
            )
            if two:
                nc.tensor.matmul(
                    out=plog[64:128, :w],
                    lhsT=ones[:],
                    rhs=ad_sb[:, pos0b : pos0b + n]
                    .unsqueeze(2)
                    .broadcast_to([1, n, d]),
                    start=False,
                    stop=True,
                )
            np_ = 128 if two else 64

            pt = sb.tile([128, TILE_W], _bf16, tag="p")
            pt2 = sb.tile([128, TILE_W], _bf16, tag="p2")
            nc.scalar.activation(
                out=pt[:np_, :w],
                in_=plog[:np_, :w],
                func=mybir.ActivationFunctionType.Exp,
            )
            nc.scalar.activation(
                out=pt2[:np_, :w],
                in_=plog[:np_, :w],
                func=mybir.ActivationFunctionType.Exp,
                scale=ATT_SLOPE,
            )
            nc.vector.tensor_max(
                out=pt[:np_, :w], in0=pt[:np_, :w], in1=pt2[:np_, :w]
            )
            mp = sb.tile([128, TILE_W], _bf16, tag="mp")
            nc.vector.tensor_mul(out=mp[:np_, :w], in0=pmsg[:np_, :w], in1=pt[:np_, :w])
            nc.vector.tensor_reduce(
                out=outacc[:np_, oc : oc + n],
                in_=mp[:np_, :w].rearrange("p (n d) -> p n d", d=d),
                axis=mybir.AxisListType.X,
                op=mybir.AluOpType.max,
            )
            nc.vector.tensor_reduce(
                out=sacc[:np_, oc : oc + n],
                in_=pt[:np_, :w].rearrange("p (n d) -> p n d", d=d),
                axis=mybir.AxisListType.X,
                op=mybir.AluOpType.add,
            )
            if not two:
                nc.vector.memset(outacc[64:128, oc : oc + n], 0.0)
                nc.vector.memset(sacc[64:128, oc : oc + n], 1.0)

        rs = acc.tile([128, NCOL], _f32)
        nc.vector.reciprocal(out=rs[:], in_=sacc[:])
        nc.vector.tensor_mul(out=outacc[:], in0=outacc[:], in1=rs[:])
        mask = acc.tile([128, NCOL], _f32)
        nc.vector.tensor_scalar(
            out=mask[:],
            in0=outacc[:],
            scalar1=float(EMPTY_THR),
            scalar2=None,
            op0=mybir.AluOpType.is_ge,
        )
        nc.vector.tensor_mul(out=outacc[:], in0=outacc[:], in1=mask[:])
        nc.vector.tensor_scalar(
            out=outacc[:],
            in0=outacc[:],
            scalar1=bvec[:],
            scalar2=None,
            op0=mybir.AluOpType.add,
        )
        nc.vector.scalar_tensor_tensor(
            out=outacc[:],
            in0=outacc[:],
            scalar=ACT_SLOPE,
            in1=outacc[:],
            op0=mybir.AluOpType.mult,
            op1=mybir.AluOpType.max,
        )
        nc.sync.dma_start(out=out_d[:], in_=outacc[:])

    nc.compile()
    return nc


def make_lhs(W, We, a_s, a_e):
    lmsg = np.zeros((K_RHS, DOUT), np.float32)
    lmsg[:DIN] = W
    lmsg[ROW_EA : ROW_EA + DE] = We
    lmsg[ROW_PAD, :] = BIG_NEG
    llog = np.zeros((K_RHS, DOUT), np.float32)
    llog[:DIN] = (W @ a_s)[:, None]
    llog[ROW_EA : ROW_EA + DE] = (We @ a_e)[:, None]
    llog[ROW_PAD, :] = PAD_LOGIT
    return lmsg, llog


def assemble(plan, outs):
    full = np.zeros((N, DOUT), np.float32)
    for pi, (ta, tb) in enumerate(plan.pairs):
        pos0, n, d = plan.tiles[ta]
        oc = int(plan.outcol[pi])
        for c in range(NC):
            nodes = plan.node_map[c, pos0 : pos0 + n]
            full[nodes] = outs[c, 0:64, oc : oc + n].T
            if tb >= 0:
                pos0b, nb, _ = plan.tiles[tb]
                nodesb = plan.node_map[c, pos0b : pos0b + nb]
                full[nodesb] = outs[c, 64:128, oc : oc + n].T
    return full


def kernel(
    X,
    edge_index,
    edge_attr,
    W1,
    We1,
    as1,
    ad1,
    ae1,
    b1,
    W2,
    We2,
    as2,
    ad2,
    ae2,
    b2,
):
    trace = os.environ.get("GAT_TRACE") == "1"
    if trace:
        _install_ntff_shim()
    LAST_EXEC_NS.clear()
    X = np.asarray(X, np.float32)
    edge_attr = np.asarray(edge_attr, np.float32)
    src = np.asarray(edge_index[0], np.int64)
    dst = np.asarray(edge_index[1], np.int64)
    W1, We1, as1, ad1, ae1, b1 = [
        np.asarray(a, np.float32) for a in (W1, We1, as1, ad1, ae1, b1)
    ]
    W2, We2, as2, ad2, ae2, b2 = [
        np.asarray(a, np.float32) for a in (W2, We2, as2, ad2, ae2, b2)
    ]

    plan = make_plan(dst)
    slot_src, slot_eid = make_slot_maps(plan, src, dst)

    valid_e = slot_eid >= 0
    ea = edge_attr[np.where(valid_e, slot_eid, 0)]
    ea[~valid_e] = 0.0
    ea_part = np.zeros((NC, DE + 1, plan.S), np.float32)
    ea_part[:, :DE, :] = ea.transpose(0, 2, 1)
    ea_part[:, DE, :] = (~valid_e).astype(np.float32)
    del ea

    nc_prog = build_program(plan)

    valid_s = slot_src >= 0

    def layer(node_feat, W, We, a_s, a_e, a_d, b):
        rhs = np.zeros((NC, K_RHS, plan.S), np.float32)
        xs = node_feat[np.where(valid_s, slot_src, 0)]
        xs[~valid_s] = 0.0
        rhs[:, :DIN, :] = xs.transpose(0, 2, 1)
        rhs[:, ROW_EA : ROW_EA + DE + 1, :] = ea_part
        xperm = node_feat[plan.node_map].transpose(0, 2, 1)
        lmsg, llog = make_lhs(W, We, a_s, a_e)
        wad = (W @ a_d)[:, None]
        bvec = np.concatenate([b, b]).reshape(128, 1).astype(np.float32)
        rhs16, xperm16 = _bf(rhs), np.ascontiguousarray(_bf(xperm))
        in_maps = [
            {
                "rhs": rhs16[c],
                "xperm": xperm16[c],
                "lmsg": _bf(lmsg),
                "llog": _bf(llog),
                "wad": _bf(wad),
                "bvec": bvec,
                "ones": np.ones((1, DOUT), ml_dtypes.bfloat16),
            }
            for c in range(NC)
        ]
        res = run_bass_kernel_spmd(
            nc_prog, in_maps, core_ids=list(range(NC)), trace=trace
        )
        if trace and res.exec_time_ns:
            LAST_EXEC_NS.append(res.exec_time_ns)
        if os.environ.get("GAT_DUMP_TRACE") == "1" and res.instructions_and_trace:
            import pickle

            def _s(v):
                return v() if callable(v) else v

            rows = [
                (str(i.engine), str(_s(i.op_name)), i.timestamp, i.duration,
                 i.evt_wait_time, str(_s(i.name)))
                for i in res.instructions_and_trace[0]
            ]
            with open(f"/tmp/gat_insts_{len(LAST_EXEC_NS)}.pkl", "wb") as f:
                pickle.dump(rows, f)
        outs = np.stack([res.results[c]["out"] for c in range(NC)])
        return assemble(plan, outs)

    c1 = layer(X, W1, We1, as1, ae1, ad1, b1)
    c2 = layer(c1, W2, We2, as2, ae2, ad2, b2)
    return c2


# revision 16
# speedup vs baseline: 1.4684x; 1.0247x over previous
"""2-layer GAT (edge features, softmax attention over dst, max aggregation)
on 8 TRN2 NeuronCores — dst-sharded, edge-slot streaming formulation.

Original staged baseline (HW exec ~1.03ms). Kept as fallback.
"""

import os
import numpy as np
import ml_dtypes
from contextlib import ExitStack

import concourse.bacc as bacc
import concourse.bass as bass
import concourse.mybir as mybir
import concourse.tile as tile
from concourse.bass_utils import run_bass_kernel_spmd

N = 50000
E = 1600000
DIN = 64
DOUT = 64
DE = 16
NC = 8
NPC = N // NC
ATT_SLOPE = 0.2
ACT_SLOPE = 0.01
PAD_LOGIT = -150.0
BIG_NEG = -1.0e30
EMPTY_THR = -1.0e6
K_RHS = DIN + DE + 1  # 81: x(0:64), ea(64:80), pad(80)
ROW_EA = DIN
ROW_PAD = DIN + DE
CHUNK_COLS = 8192
TILE_W = 512

LAST_EXEC_NS = []

_bf16 = mybir.dt.bfloat16
_f32 = mybir.dt.float32


def _bf(a):
    return np.asarray(a, np.float32).astype(ml_dtypes.bfloat16)


def _install_ntff_shim():
    import sys, types

    if "antenv.axon_hooks" in sys.modules:
        return
    try:
        sys.path.insert(0, "/root/.axon_site")
        from trn_agent_boot.trn_boot import _ntff_profile_via_ctypes

        hook = _ntff_profile_via_ctypes("/opt/axon/libaxon_pjrt.so")
        mod = types.ModuleType("antenv.axon_hooks")
        mod._hook = hook
        mod.get_axon_ntff_profile_hook = lambda: mod._hook
        mod.set_axon_ntff_profile_hook = lambda h: setattr(mod, "_hook", h)
        import antenv

        antenv.axon_hooks = mod
        sys.modules["antenv.axon_hooks"] = mod
    except Exception:
        pass


class Plan:
    pass


def make_plan(dst):
    deg = np.bincount(dst, minlength=N)
    assert deg.max() <= TILE_W, f"degree {deg.max()} > {TILE_W} unsupported"
    order = np.argsort(-deg, kind="stable")
    node_map = order.reshape(NPC, NC).T.copy()  # [NC, NPC]
    deg_map = deg[node_map]

    tiles = []  # (pos0, n, d)
    pos = 0
    while pos < NPC:
        d = max(int(deg_map[:, pos].max()), 1)
        n = min(TILE_W // d, NPC - pos)
        tiles.append((pos, n, d))
        pos += n

    pairs = []  # (ta, tb) tb=-1 for singleton
    i = 0
    while i < len(tiles):
        if (
            i + 1 < len(tiles)
            and tiles[i][1] == tiles[i + 1][1]
            and tiles[i][2] == tiles[i + 1][2]
        ):
            pairs.append((i, i + 1))
            i += 2
        else:
            pairs.append((i, -1))
            i += 1

    widths = [n * d for (_, n, d) in tiles]
    colstart = np.concatenate([[0], np.cumsum(widths)]).astype(np.int64)
    S = int(colstart[-1])

    outcol = []
    c = 0
    for a, b in pairs:
        outcol.append(c)
        c += tiles[a][1]

    classes = []
    i = 0
    while i < len(tiles):
        j = i
        while j < len(tiles) and tiles[j][2] == tiles[i][2]:
            j += 1
        classes.append((i, j, tiles[i][2]))
        i = j

    chunks = []
    plo, clo = 0, 0
    for pi, (a, b) in enumerate(pairs):
        chi = int(colstart[(b if b >= 0 else a) + 1])
        if chi - clo > CHUNK_COLS and pi > plo:
            cmid = int(colstart[pairs[pi][0]])
            chunks.append((plo, pi, clo, cmid))
            plo, clo = pi, cmid
    chunks.append((plo, len(pairs), clo, S))
    pair_chunk = {}
    for ci, (a, b, _, _) in enumerate(chunks):
        for pi in range(a, b):
            pair_chunk[pi] = ci

    p = Plan()
    p.deg, p.node_map, p.deg_map = deg, node_map, deg_map
    p.tiles, p.pairs, p.colstart, p.S = tiles, pairs, colstart, S
    p.outcol, p.NCOL, p.classes = np.array(outcol), c, classes
    p.chunks, p.pair_chunk = chunks, pair_chunk
    return p


def make_slot_maps(plan, src, dst):
    deg = plan.deg
    eorder = np.argsort(dst, kind="stable")
    starts = np.concatenate([[0], np.cumsum(deg)]).astype(np.int64)

    slot_src = np.full((NC, plan.S), -1, np.int64)
    slot_eid = np.full((NC, plan.S), -1, np.int64)
    for ti, (pos0, n, d) in enumerate(plan.tiles):
        c0 = int(plan.colstart[ti])
        nodes = plan.node_map[:, pos0 : pos0 + n]
        degs = plan.deg_map[:, pos0 : pos0 + n]
        st = starts[nodes]
        dgrid = np.arange(d)
        eidx = st[:, :, None] + dgrid[None, None, :]
        valid = dgrid[None, None, :] < degs[:, :, None]
        eidx = np.where(valid, eidx, 0)
        eids = eorder[eidx]
        slot_eid[:, c0 : c0 + n * d] = np.where(valid, eids, -1).reshape(NC, n * d)
        slot_src[:, c0 : c0 + n * d] = np.where(valid, src[eids], -1).reshape(
            NC, n * d
        )
    return slot_src, slot_eid


def build_program(plan):
    nc = bacc.Bacc("TRN2", target_bir_lowering=False, debug=False)
    S, NCOL = plan.S, plan.NCOL

    rhs_d = nc.dram_tensor("rhs", [K_RHS, S], _bf16, kind="ExternalInput")
    xperm_d = nc.dram_tensor("xperm", [DIN, NPC], _bf16, kind="ExternalInput")
    lmsg_d = nc.dram_tensor("lmsg", [K_RHS, DOUT], _bf16, kind="ExternalInput")
    llog_d = nc.dram_tensor("llog", [K_RHS, DOUT], _bf16, kind="ExternalInput")
    wad_d = nc.dram_tensor("wad", [DIN, 1], _bf16, kind="ExternalInput")
    bvec_d = nc.dram_tensor("bvec", [128, 1], _f32, kind="ExternalInput")
    ones_d = nc.dram_tensor("ones", [1, DOUT], _bf16, kind="ExternalInput")
    out_d = nc.dram_tensor("out", [128, NCOL], _f32, kind="ExternalOutput")

    with tile.TileContext(nc) as tc, ExitStack() as ctx:
        const = ctx.enter_context(tc.tile_pool(name="const", bufs=1))
        sb = ctx.enter_context(tc.tile_pool(name="sb", bufs=6))
        ps = ctx.enter_context(tc.tile_pool(name="ps", bufs=3, space="PSUM"))
        acc = ctx.enter_context(tc.tile_pool(name="acc", bufs=1))
        psa = ctx.enter_context(tc.tile_pool(name="psa", bufs=2, space="PSUM"))

        lmsg = const.tile([K_RHS, DOUT], _bf16)
        llog = const.tile([K_RHS, DOUT], _bf16)
        wad = const.tile([DIN, 1], _bf16)
        bvec = const.tile([128, 1], _f32)
        ones = const.tile([1, DOUT], _bf16)
        nc.sync.dma_start(out=ones[:], in_=ones_d[:])
        nc.sync.dma_start(out=lmsg[:], in_=lmsg_d[:])
        nc.sync.dma_start(out=llog[:], in_=llog_d[:])
        nc.sync.dma_start(out=wad[:], in_=wad_d[:])
        nc.sync.dma_start(out=bvec[:], in_=bvec_d[:])

        xperm = const.tile([DIN, NPC], _bf16)
        nc.sync.dma_start(out=xperm[:], in_=xperm_d[:])
        ad_sb = const.tile([1, NPC], _bf16)
        for j0 in range(0, NPC, TILE_W):
            w = min(TILE_W, NPC - j0)
            ap_ = psa.tile([1, TILE_W], _f32, tag="adps")
            nc.tensor.matmul(
                out=ap_[:, :w],
                lhsT=wad[:],
                rhs=xperm[:, j0 : j0 + w],
                start=True,
                stop=True,
            )
            nc.vector.tensor_copy(out=ad_sb[:, j0 : j0 + w], in_=ap_[:, :w])

        outacc = acc.tile([128, NCOL], _f32)
        sacc = acc.tile([128, NCOL], _f32)

        stage = {}
        for pi, (ta, tb) in enumerate(plan.pairs):
            pos0, n, d = plan.tiles[ta]
            w = n * d
            c0 = int(plan.colstart[ta])
            oc = int(plan.outcol[pi])
            two = tb >= 0
            wtot = 2 * w if two else w

            ci = plan.pair_chunk[pi]
            if ci not in stage:
                plo, phi, clo, chi = plan.chunks[ci]
                st = sb.tile([K_RHS, CHUNK_COLS], _bf16, tag="stage")
                dma_eng = nc.sync if ci % 2 == 0 else nc.scalar
                dma_eng.dma_start(out=st[:, : chi - clo], in_=rhs_d[:, clo:chi])
                stage = {ci: (st, clo)}
            st, clo = stage[ci]
            s0 = c0 - clo
            rt = st[:, s0 : s0 + wtot]

            pmsg = ps.tile([128, TILE_W], _f32, tag="pmsg")
            plog = ps.tile([128, TILE_W], _f32, tag="plog")
            pos0b = plan.tiles[tb][0] if two else 0
            nc.tensor.matmul(
                out=pmsg[0:64, :w], lhsT=lmsg[:], rhs=rt[:, :w], start=True, stop=True
            )
            if two:
                nc.tensor.matmul(
                    out=pmsg[64:128, :w],
                    lhsT=lmsg[:],
                    rhs=rt[:, w : 2 * w],
                    start=True,
                    stop=True,
                )
            nc.tensor.matmul(
                out=plog[0:64, :w], lhsT=llog[:], rhs=rt[:, :w], start=True, stop=False
            )
            if two:
                nc.tensor.matmul(
                    out=plog[64:128, :w],
                    lhsT=llog[:],
                    rhs=rt[:, w : 2 * w],
                    start=True,
                    stop=False,
                )
            nc.tensor.matmul(
                out=plog[0:64, :w],
                lhsT=ones[:],
                rhs=ad_sb[:, pos0 : pos0 + n].unsqueeze(2).broadcast_to([1, n, d]),
                start=False,
                stop=True,
            )
            if two:
                nc.tensor.matmul(
                    out=plog[64:128, :w],
                    lhsT=ones[:],
                    rhs=ad_sb[:, pos0b : pos0b + n]
                    .unsqueeze(2)
                    .broadcast_to([1, n, d]),
                    start=False,
                    stop=True,
                )
            np_ = 128 if two else 64

            pt = sb.tile([128, TILE_W], _bf16, tag="p")
            pt2 = sb.tile([128, TILE_W], _bf16, tag="p2")
            nc.scalar.activation(
                out=pt[:np_, :w],
                in_=plog[:np_, :w],
                func=mybir.ActivationFunctionType.Exp,
            )
            nc.scalar.activation(
                out=pt2[:np_, :w],
                in_=plog[:np_, :w],
                func=mybir.ActivationFunctionType.Exp,
                scale=ATT_SLOPE,
            )
            nc.vector.tensor_max(
                out=pt[:np_, :w], in0=pt[:np_, :w], in1=pt2[:np_, :w]
            )
            mp = sb.tile([128, TILE_W], _bf16, tag="mp")
            nc.vector.tensor_mul(out=mp[:np_, :w], in0=pmsg[:np_, :w], in1=pt[:np_, :w])
            nc.vector.tensor_reduce(
                out=outacc[:np_, oc : oc + n],
                in_=mp[:np_, :w].rearrange("p (n d) -> p n d", d=d),
                axis=mybir.AxisListType.X,
                op=mybir.AluOpType.max,
            )
            nc.vector.tensor_reduce(
                out=sacc[:np_, oc : oc + n],
                in_=pt[:np_, :w].rearrange("p (n d) -> p n d", d=d),
                axis=mybir.AxisListType.X,
                op=mybir.AluOpType.add,
            )
            if not two:
                nc.vector.memset(outacc[64:128, oc : oc + n], 0.0)
                nc.vector.memset(sacc[64:128, oc : oc + n], 1.0)

        rs = acc.tile([128, NCOL], _f32)
        nc.vector.reciprocal_approx_fast(out=rs[:], in_=sacc[:])
        nc.vector.tensor_mul(out=outacc[:], in0=outacc[:], in1=rs[:])
        mask = acc.tile([128, NCOL], _f32)
        nc.vector.scalar_tensor_tensor(
            out=outacc[:],
            in0=outacc[:],
            scalar=float(EMPTY_THR),
            in1=outacc[:],
            op0=mybir.AluOpType.is_ge,
            op1=mybir.AluOpType.mult,
        )
        nc.vector.tensor_scalar(
            out=outacc[:],
            in0=outacc[:],
            scalar1=bvec[:],
            scalar2=None,
            op0=mybir.AluOpType.add,
        )
        nc.vector.scalar_tensor_tensor(
            out=outacc[:],
            in0=outacc[:],
            scalar=ACT_SLOPE,
            in1=outacc[:],
            op0=mybir.AluOpType.mult,
            op1=mybir.AluOpType.max,
        )
        nc.sync.dma_start(out=out_d[:], in_=outacc[:])

    nc.compile()
    return nc


def make_lhs(W, We, a_s, a_e):
    lmsg = np.zeros((K_RHS, DOUT), np.float32)
    lmsg[:DIN] = W
    lmsg[ROW_EA : ROW_EA + DE] = We
    lmsg[ROW_PAD, :] = BIG_NEG
    llog = np.zeros((K_RHS, DOUT), np.float32)
    llog[:DIN] = (W @ a_s)[:, None]
    llog[ROW_EA : ROW_EA + DE] = (We @ a_e)[:, None]
    llog[ROW_PAD, :] = PAD_LOGIT
    return lmsg, llog


def assemble(plan, outs):
    full = np.zeros((N, DOUT), np.float32)
    for pi, (ta, tb) in enumerate(plan.pairs):
        pos0, n, d = plan.tiles[ta]
        oc = int(plan.outcol[pi])
        for c in range(NC):
            nodes = plan.node_map[c, pos0 : pos0 + n]
            full[nodes] = outs[c, 0:64, oc : oc + n].T
            if tb >= 0:
                pos0b, nb, _ = plan.tiles[tb]
                nodesb = plan.node_map[c, pos0b : pos0b + nb]
                full[nodesb] = outs[c, 64:128, oc : oc + n].T
    return full


def kernel(
    X,
    edge_index,
    edge_attr,
    W1,
    We1,
    as1,
    ad1,
    ae1,
    b1,
    W2,
    We2,
    as2,
    ad2,
    ae2,
    b2,
):
    trace = os.environ.get("GAT_TRACE") == "1"
    if trace:
        _install_ntff_shim()
    LAST_EXEC_NS.clear()
    X = np.asarray(X, np.float32)
    edge_attr = np.asarray(edge_attr, np.float32)
    src = np.asarray(edge_index[0], np.int64)
    dst = np.asarray(edge_index[1], np.int64)
    W1, We1, as1, ad1, ae1, b1 = [
        np.asarray(a, np.float32) for a in (W1, We1, as1, ad1, ae1, b1)
    ]
    W2, We2, as2, ad2, ae2, b2 = [
        np.asarray(a, np.float32) for a in (W2, We2, as2, ad2, ae2, b2)
    ]

    plan = make_plan(dst)
    slot_src, slot_eid = make_slot_maps(plan, src, dst)

    valid_e = slot_eid >= 0
    ea = edge_attr[np.where(valid_e, slot_eid, 0)]
    ea[~valid_e] = 0.0
    ea_part = np.zeros((NC, DE + 1, plan.S), np.float32)
    ea_part[:, :DE, :] = ea.transpose(0, 2, 1)
    ea_part[:, DE, :] = (~valid_e).astype(np.float32)
    del ea

    nc_prog = build_program(plan)

    valid_s = slot_src >= 0

    def layer(node_feat, W, We, a_s, a_e, a_d, b):
        rhs = np.zeros((NC, K_RHS, plan.S), np.float32)
        xs = node_feat[np.where(valid_s, slot_src, 0)]
        xs[~valid_s] = 0.0
        rhs[:, :DIN, :] = xs.transpose(0, 2, 1)
        rhs[:, ROW_EA : ROW_EA + DE + 1, :] = ea_part
        xperm = node_feat[plan.node_map].transpose(0, 2, 1)
        lmsg, llog = make_lhs(W, We, a_s, a_e)
        wad = (W @ a_d)[:, None]
        bvec = np.concatenate([b, b]).reshape(128, 1).astype(np.float32)
        rhs16, xperm16 = _bf(rhs), np.ascontiguousarray(_bf(xperm))
        in_maps = [
            {
                "rhs": rhs16[c],
                "xperm": xperm16[c],
                "lmsg": _bf(lmsg),
                "llog": _bf(llog),
                "wad": _bf(wad),
                "bvec": bvec,
                "ones": np.ones((1, DOUT), ml_dtypes.bfloat16),
            }
            for c in range(NC)
        ]
        res = run_bass_kernel_spmd(
            nc_prog, in_maps, core_ids=list(range(NC)), trace=trace
        )
        if trace and res.exec_time_ns:
            LAST_EXEC_NS.append(res.exec_time_ns)
        outs = np.stack([res.results[c]["out"] for c in range(NC)])
        return assemble(plan, outs)

    c1 = layer(X, W1, We1, as1, ae1, ad1, b1)
    c2 = layer(c1, W2, We2, as2, ae2, ad2, b2)
    return c2


# revision 17
# speedup vs baseline: 1.4799x; 1.0078x over previous
"""2-layer GAT (edge features, softmax attention over dst, max aggregation)
on 8 TRN2 NeuronCores — dst-sharded, edge-slot streaming formulation.

Original staged baseline (HW exec ~1.03ms). Kept as fallback.
"""

import os
import numpy as np
import ml_dtypes
from contextlib import ExitStack

import concourse.bacc as bacc
import concourse.bass as bass
import concourse.mybir as mybir
import concourse.tile as tile
from concourse.bass_utils import run_bass_kernel_spmd

N = 50000
E = 1600000
DIN = 64
DOUT = 64
DE = 16
NC = 8
NPC = N // NC
ATT_SLOPE = 0.2
ACT_SLOPE = 0.01
PAD_LOGIT = -150.0
BIG_NEG = -1.0e30
EMPTY_THR = -1.0e6
K_RHS = DIN + DE + 1  # 81: x(0:64), ea(64:80), pad(80)
ROW_EA = DIN
ROW_PAD = DIN + DE
CHUNK_COLS = 8192
TILE_W = 512

LAST_EXEC_NS = []

_bf16 = mybir.dt.bfloat16
_f32 = mybir.dt.float32


def _bf(a):
    return np.asarray(a, np.float32).astype(ml_dtypes.bfloat16)


def _install_ntff_shim():
    import sys, types

    if "antenv.axon_hooks" in sys.modules:
        return
    try:
        sys.path.insert(0, "/root/.axon_site")
        from trn_agent_boot.trn_boot import _ntff_profile_via_ctypes

        hook = _ntff_profile_via_ctypes("/opt/axon/libaxon_pjrt.so")
        mod = types.ModuleType("antenv.axon_hooks")
        mod._hook = hook
        mod.get_axon_ntff_profile_hook = lambda: mod._hook
        mod.set_axon_ntff_profile_hook = lambda h: setattr(mod, "_hook", h)
        import antenv

        antenv.axon_hooks = mod
        sys.modules["antenv.axon_hooks"] = mod
    except Exception:
        pass


class Plan:
    pass


def make_plan(dst):
    deg = np.bincount(dst, minlength=N)
    assert deg.max() <= TILE_W, f"degree {deg.max()} > {TILE_W} unsupported"
    order = np.argsort(-deg, kind="stable")
    node_map = order.reshape(NPC, NC).T.copy()  # [NC, NPC]
    deg_map = deg[node_map]

    tiles = []  # (pos0, n, d)
    pos = 0
    while pos < NPC:
        d = max(int(deg_map[:, pos].max()), 1)
        n = min(TILE_W // d, NPC - pos)
        tiles.append((pos, n, d))
        pos += n

    pairs = []  # (ta, tb) tb=-1 for singleton
    i = 0
    while i < len(tiles):
        if (
            i + 1 < len(tiles)
            and tiles[i][1] == tiles[i + 1][1]
            and tiles[i][2] == tiles[i + 1][2]
        ):
            pairs.append((i, i + 1))
            i += 2
        else:
            pairs.append((i, -1))
            i += 1

    widths = [n * d for (_, n, d) in tiles]
    colstart = np.concatenate([[0], np.cumsum(widths)]).astype(np.int64)
    S = int(colstart[-1])

    outcol = []
    c = 0
    for a, b in pairs:
        outcol.append(c)
        c += tiles[a][1]

    classes = []
    i = 0
    while i < len(tiles):
        j = i
        while j < len(tiles) and tiles[j][2] == tiles[i][2]:
            j += 1
        classes.append((i, j, tiles[i][2]))
        i = j

    chunks = []
    plo, clo = 0, 0
    for pi, (a, b) in enumerate(pairs):
        chi = int(colstart[(b if b >= 0 else a) + 1])
        if chi - clo > CHUNK_COLS and pi > plo:
            cmid = int(colstart[pairs[pi][0]])
            chunks.append((plo, pi, clo, cmid))
            plo, clo = pi, cmid
    chunks.append((plo, len(pairs), clo, S))
    pair_chunk = {}
    for ci, (a, b, _, _) in enumerate(chunks):
        for pi in range(a, b):
            pair_chunk[pi] = ci

    p = Plan()
    p.deg, p.node_map, p.deg_map = deg, node_map, deg_map
    p.tiles, p.pairs, p.colstart, p.S = tiles, pairs, colstart, S
    p.outcol, p.NCOL, p.classes = np.array(outcol), c, classes
    p.chunks, p.pair_chunk = chunks, pair_chunk
    return p


def make_slot_maps(plan, src, dst):
    deg = plan.deg
    eorder = np.argsort(dst, kind="stable")
    starts = np.concatenate([[0], np.cumsum(deg)]).astype(np.int64)

    slot_src = np.full((NC, plan.S), -1, np.int64)
    slot_eid = np.full((NC, plan.S), -1, np.int64)
    for ti, (pos0, n, d) in enumerate(plan.tiles):
        c0 = int(plan.colstart[ti])
        nodes = plan.node_map[:, pos0 : pos0 + n]
        degs = plan.deg_map[:, pos0 : pos0 + n]
        st = starts[nodes]
        dgrid = np.arange(d)
        eidx = st[:, :, None] + dgrid[None, None, :]
        valid = dgrid[None, None, :] < degs[:, :, None]
        eidx = np.where(valid, eidx, 0)
        eids = eorder[eidx]
        slot_eid[:, c0 : c0 + n * d] = np.where(valid, eids, -1).reshape(NC, n * d)
        slot_src[:, c0 : c0 + n * d] = np.where(valid, src[eids], -1).reshape(
            NC, n * d
        )
    return slot_src, slot_eid


def build_program(plan):
    nc = bacc.Bacc("TRN2", target_bir_lowering=False, debug=False)
    S, NCOL = plan.S, plan.NCOL

    rhs_d = nc.dram_tensor("rhs", [K_RHS, S], _bf16, kind="ExternalInput")
    xperm_d = nc.dram_tensor("xperm", [DIN, NPC], _bf16, kind="ExternalInput")
    lmsg_d = nc.dram_tensor("lmsg", [K_RHS, DOUT], _bf16, kind="ExternalInput")
    llog_d = nc.dram_tensor("llog", [K_RHS, DOUT], _bf16, kind="ExternalInput")
    wad_d = nc.dram_tensor("wad", [DIN, 1], _bf16, kind="ExternalInput")
    bvec_d = nc.dram_tensor("bvec", [128, 1], _f32, kind="ExternalInput")
    ones_d = nc.dram_tensor("ones", [1, DOUT], _bf16, kind="ExternalInput")
    out_d = nc.dram_tensor("out", [128, NCOL], _f32, kind="ExternalOutput")

    with tile.TileContext(nc) as tc, ExitStack() as ctx:
        const = ctx.enter_context(tc.tile_pool(name="const", bufs=1))
        sb = ctx.enter_context(tc.tile_pool(name="sb", bufs=6))
        ps = ctx.enter_context(tc.tile_pool(name="ps", bufs=3, space="PSUM"))
        acc = ctx.enter_context(tc.tile_pool(name="acc", bufs=1))
        psa = ctx.enter_context(tc.tile_pool(name="psa", bufs=2, space="PSUM"))

        lmsg = const.tile([K_RHS, DOUT], _bf16)
        llog = const.tile([K_RHS, DOUT], _bf16)
        wad = const.tile([DIN, 1], _bf16)
        bvec = const.tile([128, 1], _f32)
        ones = const.tile([1, DOUT], _bf16)
        nc.sync.dma_start(out=ones[:], in_=ones_d[:])
        nc.sync.dma_start(out=lmsg[:], in_=lmsg_d[:])
        nc.sync.dma_start(out=llog[:], in_=llog_d[:])
        nc.sync.dma_start(out=wad[:], in_=wad_d[:])
        nc.sync.dma_start(out=bvec[:], in_=bvec_d[:])

        xperm = const.tile([DIN, NPC], _bf16)
        nc.sync.dma_start(out=xperm[:], in_=xperm_d[:])
        ad_sb = const.tile([1, NPC], _bf16)
        for j0 in range(0, NPC, TILE_W):
            w = min(TILE_W, NPC - j0)
            ap_ = psa.tile([1, TILE_W], _f32, tag="adps")
            nc.tensor.matmul(
                out=ap_[:, :w],
                lhsT=wad[:],
                rhs=xperm[:, j0 : j0 + w],
                start=True,
                stop=True,
            )
            nc.vector.tensor_copy(out=ad_sb[:, j0 : j0 + w], in_=ap_[:, :w])

        outacc = acc.tile([128, NCOL], _f32)
        sacc = acc.tile([128, NCOL], _f32)

        stage = {}
        for pi, (ta, tb) in enumerate(plan.pairs):
            pos0, n, d = plan.tiles[ta]
            w = n * d
            c0 = int(plan.colstart[ta])
            oc = int(plan.outcol[pi])
            two = tb >= 0
            wtot = 2 * w if two else w

            ci = plan.pair_chunk[pi]
            if ci not in stage:
                plo, phi, clo, chi = plan.chunks[ci]
                st = sb.tile([K_RHS, CHUNK_COLS], _bf16, tag="stage")
                dma_eng = nc.sync if ci % 2 == 0 else nc.scalar
                dma_eng.dma_start(out=st[:, : chi - clo], in_=rhs_d[:, clo:chi])
                stage = {ci: (st, clo)}
            st, clo = stage[ci]
            s0 = c0 - clo
            rt = st[:, s0 : s0 + wtot]

            pmsg = ps.tile([128, TILE_W], _f32, tag="pmsg")
            plog = ps.tile([128, TILE_W], _f32, tag="plog")
            pos0b = plan.tiles[tb][0] if two else 0
            nc.tensor.matmul(
                out=pmsg[0:64, :w], lhsT=lmsg[:], rhs=rt[:, :w], start=True, stop=True
            )
            if two:
                nc.tensor.matmul(
                    out=pmsg[64:128, :w],
                    lhsT=lmsg[:],
                    rhs=rt[:, w : 2 * w],
                    start=True,
                    stop=True,
                )
            nc.tensor.matmul(
                out=plog[0:64, :w], lhsT=llog[:], rhs=rt[:, :w], start=True, stop=False
            )
            if two:
                nc.tensor.matmul(
                    out=plog[64:128, :w],
                    lhsT=llog[:],
                    rhs=rt[:, w : 2 * w],
                    start=True,
                    stop=False,
                )
            nc.tensor.matmul(
                out=plog[0:64, :w],
                lhsT=ones[:],
                rhs=ad_sb[:, pos0 : pos0 + n].unsqueeze(2).broadcast_to([1, n, d]),
                start=False,
                stop=True,
            )
            if two:
                nc.tensor.matmul(
                    out=plog[64:128, :w],
                    lhsT=ones[:],
                    rhs=ad_sb[:, pos0b : pos0b + n]
                    .unsqueeze(2)
                    .broadcast_to([1, n, d]),
                    start=False,
                    stop=True,
                )
            np_ = 128 if two else 64

            pt = sb.tile([128, TILE_W], _bf16, tag="p")
            pt2 = sb.tile([128, TILE_W], _bf16, tag="p2")
            mp = sb.tile([128, TILE_W], _bf16, tag="mp")
            nc.scalar.activation(
                out=mp[:np_, :w],
                in_=pmsg[:np_, :w],
                func=mybir.ActivationFunctionType.Copy,
            )
            nc.scalar.activation(
                out=pt[:np_, :w],
                in_=plog[:np_, :w],
                func=mybir.ActivationFunctionType.Exp,
            )
            nc.scalar.activation(
                out=pt2[:np_, :w],
                in_=plog[:np_, :w],
                func=mybir.ActivationFunctionType.Exp,
                scale=ATT_SLOPE,
            )
            nc.vector.tensor_max(
                out=pt[:np_, :w], in0=pt[:np_, :w], in1=pt2[:np_, :w]
            )
            nc.vector.tensor_mul(out=mp[:np_, :w], in0=mp[:np_, :w], in1=pt[:np_, :w])
            nc.vector.tensor_reduce(
                out=outacc[:np_, oc : oc + n],
                in_=mp[:np_, :w].rearrange("p (n d) -> p n d", d=d),
                axis=mybir.AxisListType.X,
                op=mybir.AluOpType.max,
            )
            nc.vector.tensor_reduce(
                out=sacc[:np_, oc : oc + n],
                in_=pt[:np_, :w].rearrange("p (n d) -> p n d", d=d),
                axis=mybir.AxisListType.X,
                op=mybir.AluOpType.add,
            )
            if not two:
                nc.vector.memset(outacc[64:128, oc : oc + n], 0.0)
                nc.vector.memset(sacc[64:128, oc : oc + n], 1.0)

        rs = acc.tile([128, NCOL], _f32)
        nc.vector.reciprocal_approx_fast(out=rs[:], in_=sacc[:])
        nc.vector.tensor_mul(out=outacc[:], in0=outacc[:], in1=rs[:])
        mask = acc.tile([128, NCOL], _f32)
        nc.vector.scalar_tensor_tensor(
            out=outacc[:],
            in0=outacc[:],
            scalar=float(EMPTY_THR),
            in1=outacc[:],
            op0=mybir.AluOpType.is_ge,
            op1=mybir.AluOpType.mult,
        )
        nc.vector.tensor_scalar(
            out=outacc[:],
            in0=outacc[:],
            scalar1=bvec[:],
            scalar2=None,
            op0=mybir.AluOpType.add,
        )
        nc.vector.scalar_tensor_tensor(
            out=outacc[:],
            in0=outacc[:],
            scalar=ACT_SLOPE,
            in1=outacc[:],
            op0=mybir.AluOpType.mult,
            op1=mybir.AluOpType.max,
        )
        nc.sync.dma_start(out=out_d[:], in_=outacc[:])

    nc.compile()
    return nc


def make_lhs(W, We, a_s, a_e):
    lmsg = np.zeros((K_RHS, DOUT), np.float32)
    lmsg[:DIN] = W
    lmsg[ROW_EA : ROW_EA + DE] = We
    lmsg[ROW_PAD, :] = BIG_NEG
    llog = np.zeros((K_RHS, DOUT), np.float32)
    llog[:DIN] = (W @ a_s)[:, None]
    llog[ROW_EA : ROW_EA + DE] = (We @ a_e)[:, None]
    llog[ROW_PAD, :] = PAD_LOGIT
    return lmsg, llog


def assemble(plan, outs):
    full = np.zeros((N, DOUT), np.float32)
    for pi, (ta, tb) in enumerate(plan.pairs):
        pos0, n, d = plan.tiles[ta]
        oc = int(plan.outcol[pi])
        for c in range(NC):
            nodes = plan.node_map[c, pos0 : pos0 + n]
            full[nodes] = outs[c, 0:64, oc : oc + n].T
            if tb >= 0:
                pos0b, nb, _ = plan.tiles[tb]
                nodesb = plan.node_map[c, pos0b : pos0b + nb]
                full[nodesb] = outs[c, 64:128, oc : oc + n].T
    return full


def kernel(
    X,
    edge_index,
    edge_attr,
    W1,
    We1,
    as1,
    ad1,
    ae1,
    b1,
    W2,
    We2,
    as2,
    ad2,
    ae2,
    b2,
):
    trace = os.environ.get("GAT_TRACE") == "1"
    if trace:
        _install_ntff_shim()
    LAST_EXEC_NS.clear()
    X = np.asarray(X, np.float32)
    edge_attr = np.asarray(edge_attr, np.float32)
    src = np.asarray(edge_index[0], np.int64)
    dst = np.asarray(edge_index[1], np.int64)
    W1, We1, as1, ad1, ae1, b1 = [
        np.asarray(a, np.float32) for a in (W1, We1, as1, ad1, ae1, b1)
    ]
    W2, We2, as2, ad2, ae2, b2 = [
        np.asarray(a, np.float32) for a in (W2, We2, as2, ad2, ae2, b2)
    ]

    plan = make_plan(dst)
    slot_src, slot_eid = make_slot_maps(plan, src, dst)

    valid_e = slot_eid >= 0
    ea = edge_attr[np.where(valid_e, slot_eid, 0)]
    ea[~valid_e] = 0.0
    ea_part = np.zeros((NC, DE + 1, plan.S), np.float32)
    ea_part[:, :DE, :] = ea.transpose(0, 2, 1)
    ea_part[:, DE, :] = (~valid_e).astype(np.float32)
    del ea

    nc_prog = build_program(plan)

    valid_s = slot_src >= 0

    def layer(node_feat, W, We, a_s, a_e, a_d, b):
        rhs = np.zeros((NC, K_RHS, plan.S), np.float32)
        xs = node_feat[np.where(valid_s, slot_src, 0)]
        xs[~valid_s] = 0.0
        rhs[:, :DIN, :] = xs.transpose(0, 2, 1)
        rhs[:, ROW_EA : ROW_EA + DE + 1, :] = ea_part
        xperm = node_feat[plan.node_map].transpose(0, 2, 1)
        lmsg, llog = make_lhs(W, We, a_s, a_e)
        wad = (W @ a_d)[:, None]
        bvec = np.concatenate([b, b]).reshape(128, 1).astype(np.float32)
        rhs16, xperm16 = _bf(rhs), np.ascontiguousarray(_bf(xperm))
        in_maps = [
            {
                "rhs": rhs16[c],
                "xperm": xperm16[c],
                "lmsg": _bf(lmsg),
                "llog": _bf(llog),
                "wad": _bf(wad),
                "bvec": bvec,
                "ones": np.ones((1, DOUT), ml_dtypes.bfloat16),
            }
            for c in range(NC)
        ]
        res = run_bass_kernel_spmd(
            nc_prog, in_maps, core_ids=list(range(NC)), trace=trace
        )
        if trace and res.exec_time_ns:
            LAST_EXEC_NS.append(res.exec_time_ns)
        outs = np.stack([res.results[c]["out"] for c in range(NC)])
        return assemble(plan, outs)

    c1 = layer(X, W1, We1, as1, ae1, ad1, b1)
    c2 = layer(c1, W2, We2, as2, ae2, ad2, b2)
    return c2


# revision 18
# speedup vs baseline: 1.6194x; 1.0943x over previous
"""2-layer GAT (edge features, softmax attention over dst, max aggregation)
on 8 TRN2 NeuronCores — dst-sharded, edge-slot streaming formulation.

Original staged baseline (HW exec ~1.03ms). Kept as fallback.
"""

import os
import numpy as np
import ml_dtypes
from contextlib import ExitStack

import concourse.bacc as bacc
import concourse.bass as bass
import concourse.mybir as mybir
import concourse.tile as tile
from concourse.bass_utils import run_bass_kernel_spmd

N = 50000
E = 1600000
DIN = 64
DOUT = 64
DE = 16
NC = 8
NPC = N // NC
ATT_SLOPE = 0.2
ACT_SLOPE = 0.01
PAD_LOGIT = -150.0
BIG_NEG = -1.0e30
EMPTY_THR = -1.0e6
K_RHS = DIN + DE + 1  # 81: x(0:64), ea(64:80), pad(80)
ROW_EA = DIN
ROW_PAD = DIN + DE
CHUNK_COLS = 8192
TILE_W = 512

LAST_EXEC_NS = []

_bf16 = mybir.dt.bfloat16
_f32 = mybir.dt.float32


def _bf(a):
    return np.asarray(a, np.float32).astype(ml_dtypes.bfloat16)


def _install_ntff_shim():
    import sys, types

    if "antenv.axon_hooks" in sys.modules:
        return
    try:
        sys.path.insert(0, "/root/.axon_site")
        from trn_agent_boot.trn_boot import _ntff_profile_via_ctypes

        hook = _ntff_profile_via_ctypes("/opt/axon/libaxon_pjrt.so")
        mod = types.ModuleType("antenv.axon_hooks")
        mod._hook = hook
        mod.get_axon_ntff_profile_hook = lambda: mod._hook
        mod.set_axon_ntff_profile_hook = lambda h: setattr(mod, "_hook", h)
        import antenv

        antenv.axon_hooks = mod
        sys.modules["antenv.axon_hooks"] = mod
    except Exception:
        pass


class Plan:
    pass


def make_plan(dst):
    deg = np.bincount(dst, minlength=N)
    assert deg.max() <= TILE_W, f"degree {deg.max()} > {TILE_W} unsupported"
    order = np.argsort(-deg, kind="stable")
    node_map = order.reshape(NPC, NC).T.copy()  # [NC, NPC]
    deg_map = deg[node_map]

    tiles = []  # (pos0, n, d)
    pos = 0
    while pos < NPC:
        d = max(int(deg_map[:, pos].max()), 1)
        n = min(TILE_W // d, NPC - pos)
        tiles.append((pos, n, d))
        pos += n

    pairs = []  # (ta, tb) tb=-1 for singleton
    i = 0
    while i < len(tiles):
        if (
            i + 1 < len(tiles)
            and tiles[i][1] == tiles[i + 1][1]
            and tiles[i][2] == tiles[i + 1][2]
        ):
            pairs.append((i, i + 1))
            i += 2
        else:
            pairs.append((i, -1))
            i += 1

    widths = [n * d for (_, n, d) in tiles]
    colstart = np.concatenate([[0], np.cumsum(widths)]).astype(np.int64)
    S = int(colstart[-1])

    outcol = []
    c = 0
    for a, b in pairs:
        outcol.append(c)
        c += tiles[a][1]

    classes = []
    i = 0
    while i < len(tiles):
        j = i
        while j < len(tiles) and tiles[j][2] == tiles[i][2]:
            j += 1
        classes.append((i, j, tiles[i][2]))
        i = j

    chunks = []
    plo, clo = 0, 0
    for pi, (a, b) in enumerate(pairs):
        chi = int(colstart[(b if b >= 0 else a) + 1])
        if chi - clo > CHUNK_COLS and pi > plo:
            cmid = int(colstart[pairs[pi][0]])
            chunks.append((plo, pi, clo, cmid))
            plo, clo = pi, cmid
    chunks.append((plo, len(pairs), clo, S))
    pair_chunk = {}
    for ci, (a, b, _, _) in enumerate(chunks):
        for pi in range(a, b):
            pair_chunk[pi] = ci

    p = Plan()
    p.deg, p.node_map, p.deg_map = deg, node_map, deg_map
    p.tiles, p.pairs, p.colstart, p.S = tiles, pairs, colstart, S
    p.outcol, p.NCOL, p.classes = np.array(outcol), c, classes
    p.chunks, p.pair_chunk = chunks, pair_chunk
    return p


def make_slot_maps(plan, src, dst):
    deg = plan.deg
    eorder = np.argsort(dst, kind="stable")
    starts = np.concatenate([[0], np.cumsum(deg)]).astype(np.int64)

    slot_src = np.full((NC, plan.S), -1, np.int64)
    slot_eid = np.full((NC, plan.S), -1, np.int64)
    for ti, (pos0, n, d) in enumerate(plan.tiles):
        c0 = int(plan.colstart[ti])
        nodes = plan.node_map[:, pos0 : pos0 + n]
        degs = plan.deg_map[:, pos0 : pos0 + n]
        st = starts[nodes]
        dgrid = np.arange(d)
        eidx = st[:, :, None] + dgrid[None, None, :]
        valid = dgrid[None, None, :] < degs[:, :, None]
        eidx = np.where(valid, eidx, 0)
        eids = eorder[eidx]
        slot_eid[:, c0 : c0 + n * d] = np.where(valid, eids, -1).reshape(NC, n * d)
        slot_src[:, c0 : c0 + n * d] = np.where(valid, src[eids], -1).reshape(
            NC, n * d
        )
    return slot_src, slot_eid


def build_program(plan):
    nc = bacc.Bacc("TRN2", target_bir_lowering=False, debug=False)
    S, NCOL = plan.S, plan.NCOL

    rhs_d = nc.dram_tensor("rhs", [K_RHS, S], _bf16, kind="ExternalInput")
    xperm_d = nc.dram_tensor("xperm", [DIN, NPC], _bf16, kind="ExternalInput")
    lmsg_d = nc.dram_tensor("lmsg", [K_RHS, DOUT], _bf16, kind="ExternalInput")
    llog_d = nc.dram_tensor("llog", [K_RHS, DOUT], _bf16, kind="ExternalInput")
    wad_d = nc.dram_tensor("wad", [DIN, 1], _bf16, kind="ExternalInput")
    bvec_d = nc.dram_tensor("bvec", [128, 1], _f32, kind="ExternalInput")
    ones_d = nc.dram_tensor("ones", [1, DOUT], _bf16, kind="ExternalInput")
    out_d = nc.dram_tensor("out", [128, NCOL], _f32, kind="ExternalOutput")

    with tile.TileContext(nc) as tc, ExitStack() as ctx:
        const = ctx.enter_context(tc.tile_pool(name="const", bufs=1))
        sb = ctx.enter_context(tc.tile_pool(name="sb", bufs=6))
        ps = ctx.enter_context(tc.tile_pool(name="ps", bufs=3, space="PSUM"))
        acc = ctx.enter_context(tc.tile_pool(name="acc", bufs=1))
        psa = ctx.enter_context(tc.tile_pool(name="psa", bufs=2, space="PSUM"))

        lmsg = const.tile([K_RHS, DOUT], _bf16)
        llog = const.tile([K_RHS, DOUT], _bf16)
        wad = const.tile([DIN, 1], _bf16)
        bvec = const.tile([128, 1], _f32)
        ones = const.tile([1, DOUT], _bf16)
        nc.sync.dma_start(out=ones[:], in_=ones_d[:])
        nc.sync.dma_start(out=lmsg[:], in_=lmsg_d[:])
        nc.sync.dma_start(out=llog[:], in_=llog_d[:])
        nc.sync.dma_start(out=wad[:], in_=wad_d[:])
        nc.sync.dma_start(out=bvec[:], in_=bvec_d[:])

        xperm = const.tile([DIN, NPC], _bf16)
        nc.sync.dma_start(out=xperm[:], in_=xperm_d[:])
        ad_sb = const.tile([1, NPC], _bf16)
        for j0 in range(0, NPC, TILE_W):
            w = min(TILE_W, NPC - j0)
            ap_ = psa.tile([1, TILE_W], _f32, tag="adps")
            nc.tensor.matmul(
                out=ap_[:, :w],
                lhsT=wad[:],
                rhs=xperm[:, j0 : j0 + w],
                start=True,
                stop=True,
            )
            nc.vector.tensor_copy(out=ad_sb[:, j0 : j0 + w], in_=ap_[:, :w])

        outacc = acc.tile([128, NCOL], _f32)
        sacc = acc.tile([128, NCOL], _f32)

        stage = {}
        for pi, (ta, tb) in enumerate(plan.pairs):
            pos0, n, d = plan.tiles[ta]
            w = n * d
            c0 = int(plan.colstart[ta])
            oc = int(plan.outcol[pi])
            two = tb >= 0
            wtot = 2 * w if two else w

            ci = plan.pair_chunk[pi]
            if ci not in stage:
                plo, phi, clo, chi = plan.chunks[ci]
                st = sb.tile([K_RHS, CHUNK_COLS], _bf16, tag="stage")
                dma_eng = nc.sync if ci % 2 == 0 else nc.scalar
                dma_eng.dma_start(out=st[:, : chi - clo], in_=rhs_d[:, clo:chi])
                stage = {ci: (st, clo)}
            st, clo = stage[ci]
            s0 = c0 - clo
            rt = st[:, s0 : s0 + wtot]

            pmsg = ps.tile([128, TILE_W], _f32, tag="pmsg")
            plog = ps.tile([128, TILE_W], _f32, tag="plog")
            pos0b = plan.tiles[tb][0] if two else 0
            nc.tensor.matmul(
                out=pmsg[0:64, :w], lhsT=lmsg[:], rhs=rt[:, :w], start=True, stop=True
            )
            if two:
                nc.tensor.matmul(
                    out=pmsg[64:128, :w],
                    lhsT=lmsg[:],
                    rhs=rt[:, w : 2 * w],
                    start=True,
                    stop=True,
                )
            nc.tensor.matmul(
                out=plog[0:64, :w], lhsT=llog[:], rhs=rt[:, :w], start=True, stop=False
            )
            if two:
                nc.tensor.matmul(
                    out=plog[64:128, :w],
                    lhsT=llog[:],
                    rhs=rt[:, w : 2 * w],
                    start=True,
                    stop=False,
                )
            nc.tensor.matmul(
                out=plog[0:64, :w],
                lhsT=ones[:],
                rhs=ad_sb[:, pos0 : pos0 + n].unsqueeze(2).broadcast_to([1, n, d]),
                start=False,
                stop=True,
            )
            if two:
                nc.tensor.matmul(
                    out=plog[64:128, :w],
                    lhsT=ones[:],
                    rhs=ad_sb[:, pos0b : pos0b + n]
                    .unsqueeze(2)
                    .broadcast_to([1, n, d]),
                    start=False,
                    stop=True,
                )
            np_ = 128 if two else 64

            pt = sb.tile([128, TILE_W], _bf16, tag="p")
            pt2 = sb.tile([128, TILE_W], _bf16, tag="p2")
            mp = sb.tile([128, TILE_W], _bf16, tag="mp")
            nc.scalar.activation(
                out=mp[:np_, :w],
                in_=pmsg[:np_, :w],
                func=mybir.ActivationFunctionType.Copy,
            )
            nc.scalar.activation(
                out=pt[:np_, :w],
                in_=plog[:np_, :w],
                func=mybir.ActivationFunctionType.Exp,
            )
            nc.scalar.activation(
                out=pt2[:np_, :w],
                in_=plog[:np_, :w],
                func=mybir.ActivationFunctionType.Exp,
                scale=ATT_SLOPE,
            )
            nc.vector.tensor_max(
                out=pt[:np_, :w], in0=pt[:np_, :w], in1=pt2[:np_, :w]
            )
            nc.vector.tensor_mul(out=mp[:np_, :w], in0=mp[:np_, :w], in1=pt[:np_, :w])
            nc.vector.tensor_reduce(
                out=outacc[:np_, oc : oc + n],
                in_=mp[:np_, :w].rearrange("p (n d) -> p n d", d=d),
                axis=mybir.AxisListType.X,
                op=mybir.AluOpType.max,
            )
            nc.vector.tensor_reduce(
                out=sacc[:np_, oc : oc + n],
                in_=pt[:np_, :w].rearrange("p (n d) -> p n d", d=d),
                axis=mybir.AxisListType.X,
                op=mybir.AluOpType.add,
            )
            if not two:
                nc.vector.memset(outacc[64:128, oc : oc + n], 0.0)
                nc.vector.memset(sacc[64:128, oc : oc + n], 1.0)

        rs = acc.tile([128, NCOL], _f32)
        nc.vector.reciprocal_approx_fast(out=rs[:], in_=sacc[:])
        nc.vector.tensor_mul(out=outacc[:], in0=outacc[:], in1=rs[:])
        mask = acc.tile([128, NCOL], _f32)
        nc.vector.scalar_tensor_tensor(
            out=outacc[:],
            in0=outacc[:],
            scalar=float(EMPTY_THR),
            in1=outacc[:],
            op0=mybir.AluOpType.is_ge,
            op1=mybir.AluOpType.mult,
        )
        nc.vector.tensor_scalar(
            out=outacc[:],
            in0=outacc[:],
            scalar1=bvec[:],
            scalar2=None,
            op0=mybir.AluOpType.add,
        )
        nc.vector.scalar_tensor_tensor(
            out=outacc[:],
            in0=outacc[:],
            scalar=ACT_SLOPE,
            in1=outacc[:],
            op0=mybir.AluOpType.mult,
            op1=mybir.AluOpType.max,
        )
        nc.sync.dma_start(out=out_d[:], in_=outacc[:])

    nc.compile()
    return nc


def make_lhs(W, We, a_s, a_e):
    lmsg = np.zeros((K_RHS, DOUT), np.float32)
    lmsg[:DIN] = W
    lmsg[ROW_EA : ROW_EA + DE] = We
    lmsg[ROW_PAD, :] = BIG_NEG
    llog = np.zeros((K_RHS, DOUT), np.float32)
    llog[:DIN] = (W @ a_s)[:, None]
    llog[ROW_EA : ROW_EA + DE] = (We @ a_e)[:, None]
    llog[ROW_PAD, :] = PAD_LOGIT
    return lmsg, llog


def assemble(plan, outs):
    full = np.zeros((N, DOUT), np.float32)
    for pi, (ta, tb) in enumerate(plan.pairs):
        pos0, n, d = plan.tiles[ta]
        oc = int(plan.outcol[pi])
        for c in range(NC):
            nodes = plan.node_map[c, pos0 : pos0 + n]
            full[nodes] = outs[c, 0:64, oc : oc + n].T
            if tb >= 0:
                pos0b, nb, _ = plan.tiles[tb]
                nodesb = plan.node_map[c, pos0b : pos0b + nb]
                full[nodesb] = outs[c, 64:128, oc : oc + n].T
    return full


def kernel(
    X,
    edge_index,
    edge_attr,
    W1,
    We1,
    as1,
    ad1,
    ae1,
    b1,
    W2,
    We2,
    as2,
    ad2,
    ae2,
    b2,
):
    trace = os.environ.get("GAT_TRACE") == "1"
    if trace:
        _install_ntff_shim()
    LAST_EXEC_NS.clear()
    X = np.asarray(X, np.float32)
    edge_attr = np.asarray(edge_attr, np.float32)
    src = np.asarray(edge_index[0], np.int64)
    dst = np.asarray(edge_index[1], np.int64)
    W1, We1, as1, ad1, ae1, b1 = [
        np.asarray(a, np.float32) for a in (W1, We1, as1, ad1, ae1, b1)
    ]
    W2, We2, as2, ad2, ae2, b2 = [
        np.asarray(a, np.float32) for a in (W2, We2, as2, ad2, ae2, b2)
    ]

    plan = make_plan(dst)
    slot_src, slot_eid = make_slot_maps(plan, src, dst)

    valid_e = slot_eid >= 0
    ea = edge_attr[np.where(valid_e, slot_eid, 0)]
    ea[~valid_e] = 0.0
    ea_part = np.zeros((NC, DE + 1, plan.S), np.float32)
    ea_part[:, :DE, :] = ea.transpose(0, 2, 1)
    ea_part[:, DE, :] = (~valid_e).astype(np.float32)
    del ea

    nc_prog = build_program(plan)

    valid_s = slot_src >= 0

    def layer(node_feat, W, We, a_s, a_e, a_d, b):
        rhs = np.zeros((NC, K_RHS, plan.S), np.float32)
        xs = node_feat[np.where(valid_s, slot_src, 0)]
        xs[~valid_s] = 0.0
        rhs[:, :DIN, :] = xs.transpose(0, 2, 1)
        rhs[:, ROW_EA : ROW_EA + DE + 1, :] = ea_part
        xperm = node_feat[plan.node_map].transpose(0, 2, 1)
        lmsg, llog = make_lhs(W, We, a_s, a_e)
        wad = (W @ a_d)[:, None]
        bvec = np.concatenate([b, b]).reshape(128, 1).astype(np.float32)
        rhs16, xperm16 = _bf(rhs), np.ascontiguousarray(_bf(xperm))
        in_maps = [
            {
                "rhs": rhs16[c],
                "xperm": xperm16[c],
                "lmsg": _bf(lmsg),
                "llog": _bf(llog),
                "wad": _bf(wad),
                "bvec": bvec,
                "ones": np.ones((1, DOUT), ml_dtypes.bfloat16),
            }
            for c in range(NC)
        ]
        res = run_bass_kernel_spmd(
            nc_prog, in_maps, core_ids=list(range(NC)), trace=trace
        )
        if trace and res.exec_time_ns:
            LAST_EXEC_NS.append(res.exec_time_ns)
        if os.environ.get("GAT_DUMP_TRACE") == "1" and res.instructions_and_trace:
            import pickle

            def _s(v):
                return v() if callable(v) else v

            rows = [
                (str(i.engine), str(_s(i.op_name)), i.timestamp, i.duration,
                 i.evt_wait_time, str(_s(i.name)))
                for i in res.instructions_and_trace[0]
            ]
            with open(f"/tmp/gat_insts_{len(LAST_EXEC_NS)}.pkl", "wb") as f:
                pickle.dump(rows, f)
        outs = np.stack([res.results[c]["out"] for c in range(NC)])
        return assemble(plan, outs)

    c1 = layer(X, W1, We1, as1, ae1, ad1, b1)
    c2 = layer(c1, W2, We2, as2, ae2, ad2, b2)
    return c2


# revision 19
# speedup vs baseline: 1.6246x; 1.0032x over previous
"""2-layer GAT (edge features, softmax attention over dst, max aggregation)
on 8 TRN2 NeuronCores — dst-sharded, edge-slot streaming formulation.

Original staged baseline (HW exec ~1.03ms). Kept as fallback.
"""

import os
import numpy as np
import ml_dtypes
from contextlib import ExitStack

import concourse.bacc as bacc
import concourse.bass as bass
import concourse.mybir as mybir
import concourse.tile as tile
from concourse.bass_utils import run_bass_kernel_spmd

N = 50000
E = 1600000
DIN = 64
DOUT = 64
DE = 16
NC = 8
NPC = N // NC
ATT_SLOPE = 0.2
ACT_SLOPE = 0.01
PAD_LOGIT = -150.0
BIG_NEG = -1.0e30
EMPTY_THR = -1.0e6
K_RHS = DIN + DE + 1  # 81: x(0:64), ea(64:80), pad(80)
ROW_EA = DIN
ROW_PAD = DIN + DE
CHUNK_COLS = 8192
TILE_W = 512

LAST_EXEC_NS = []

_bf16 = mybir.dt.bfloat16
_f32 = mybir.dt.float32


def _bf(a):
    return np.asarray(a, np.float32).astype(ml_dtypes.bfloat16)


def _install_ntff_shim():
    import sys, types

    if "antenv.axon_hooks" in sys.modules:
        return
    try:
        sys.path.insert(0, "/root/.axon_site")
        from trn_agent_boot.trn_boot import _ntff_profile_via_ctypes

        hook = _ntff_profile_via_ctypes("/opt/axon/libaxon_pjrt.so")
        mod = types.ModuleType("antenv.axon_hooks")
        mod._hook = hook
        mod.get_axon_ntff_profile_hook = lambda: mod._hook
        mod.set_axon_ntff_profile_hook = lambda h: setattr(mod, "_hook", h)
        import antenv

        antenv.axon_hooks = mod
        sys.modules["antenv.axon_hooks"] = mod
    except Exception:
        pass


class Plan:
    pass


def make_plan(dst):
    deg = np.bincount(dst, minlength=N)
    assert deg.max() <= TILE_W, f"degree {deg.max()} > {TILE_W} unsupported"
    order = np.argsort(-deg, kind="stable")
    node_map = order.reshape(NPC, NC).T.copy()  # [NC, NPC]
    deg_map = deg[node_map]

    tiles = []  # (pos0, n, d)
    pos = 0
    while pos < NPC:
        d = max(int(deg_map[:, pos].max()), 1)
        n = min(TILE_W // d, NPC - pos)
        tiles.append((pos, n, d))
        pos += n

    pairs = []  # (ta, tb) tb=-1 for singleton
    i = 0
    while i < len(tiles):
        if (
            i + 1 < len(tiles)
            and tiles[i][1] == tiles[i + 1][1]
            and tiles[i][2] == tiles[i + 1][2]
        ):
            pairs.append((i, i + 1))
            i += 2
        else:
            pairs.append((i, -1))
            i += 1

    widths = [n * d for (_, n, d) in tiles]
    colstart = np.concatenate([[0], np.cumsum(widths)]).astype(np.int64)
    S = int(colstart[-1])

    outcol = []
    c = 0
    for a, b in pairs:
        outcol.append(c)
        c += tiles[a][1]

    classes = []
    i = 0
    while i < len(tiles):
        j = i
        while j < len(tiles) and tiles[j][2] == tiles[i][2]:
            j += 1
        classes.append((i, j, tiles[i][2]))
        i = j

    chunks = []
    plo, clo = 0, 0
    for pi, (a, b) in enumerate(pairs):
        chi = int(colstart[(b if b >= 0 else a) + 1])
        if chi - clo > CHUNK_COLS and pi > plo:
            cmid = int(colstart[pairs[pi][0]])
            chunks.append((plo, pi, clo, cmid))
            plo, clo = pi, cmid
    chunks.append((plo, len(pairs), clo, S))
    pair_chunk = {}
    for ci, (a, b, _, _) in enumerate(chunks):
        for pi in range(a, b):
            pair_chunk[pi] = ci

    p = Plan()
    p.deg, p.node_map, p.deg_map = deg, node_map, deg_map
    p.tiles, p.pairs, p.colstart, p.S = tiles, pairs, colstart, S
    p.outcol, p.NCOL, p.classes = np.array(outcol), c, classes
    p.chunks, p.pair_chunk = chunks, pair_chunk
    return p


def make_slot_maps(plan, src, dst):
    deg = plan.deg
    eorder = np.argsort(dst, kind="stable")
    starts = np.concatenate([[0], np.cumsum(deg)]).astype(np.int64)

    slot_src = np.full((NC, plan.S), -1, np.int64)
    slot_eid = np.full((NC, plan.S), -1, np.int64)
    for ti, (pos0, n, d) in enumerate(plan.tiles):
        c0 = int(plan.colstart[ti])
        nodes = plan.node_map[:, pos0 : pos0 + n]
        degs = plan.deg_map[:, pos0 : pos0 + n]
        st = starts[nodes]
        dgrid = np.arange(d)
        eidx = st[:, :, None] + dgrid[None, None, :]
        valid = dgrid[None, None, :] < degs[:, :, None]
        eidx = np.where(valid, eidx, 0)
        eids = eorder[eidx]
        slot_eid[:, c0 : c0 + n * d] = np.where(valid, eids, -1).reshape(NC, n * d)
        slot_src[:, c0 : c0 + n * d] = np.where(valid, src[eids], -1).reshape(
            NC, n * d
        )
    return slot_src, slot_eid


def build_program(plan):
    nc = bacc.Bacc("TRN2", target_bir_lowering=False, debug=False)
    S, NCOL = plan.S, plan.NCOL

    rhs_d = nc.dram_tensor("rhs", [K_RHS, S], _bf16, kind="ExternalInput")
    xperm_d = nc.dram_tensor("xperm", [DIN, NPC], _bf16, kind="ExternalInput")
    lmsg_d = nc.dram_tensor("lmsg", [K_RHS, DOUT], _bf16, kind="ExternalInput")
    llog_d = nc.dram_tensor("llog", [K_RHS, DOUT], _bf16, kind="ExternalInput")
    wad_d = nc.dram_tensor("wad", [DIN, 1], _bf16, kind="ExternalInput")
    bvec_d = nc.dram_tensor("bvec", [128, 1], _f32, kind="ExternalInput")
    ones_d = nc.dram_tensor("ones", [1, DOUT], _bf16, kind="ExternalInput")
    out_d = nc.dram_tensor("out", [128, NCOL], _f32, kind="ExternalOutput")

    with tile.TileContext(nc) as tc, ExitStack() as ctx:
        const = ctx.enter_context(tc.tile_pool(name="const", bufs=1))
        sb = ctx.enter_context(tc.tile_pool(name="sb", bufs=6))
        ps = ctx.enter_context(tc.tile_pool(name="ps", bufs=3, space="PSUM"))
        acc = ctx.enter_context(tc.tile_pool(name="acc", bufs=1))
        psa = ctx.enter_context(tc.tile_pool(name="psa", bufs=2, space="PSUM"))

        lmsg = const.tile([K_RHS, DOUT], _bf16)
        llog = const.tile([K_RHS, DOUT], _bf16)
        wad = const.tile([DIN, 1], _bf16)
        bvec = const.tile([128, 1], _f32)
        ones = const.tile([1, DOUT], _bf16)
        nc.sync.dma_start(out=ones[:], in_=ones_d[:])
        nc.sync.dma_start(out=lmsg[:], in_=lmsg_d[:])
        nc.sync.dma_start(out=llog[:], in_=llog_d[:])
        nc.sync.dma_start(out=wad[:], in_=wad_d[:])
        nc.sync.dma_start(out=bvec[:], in_=bvec_d[:])

        xperm = const.tile([DIN, NPC], _bf16)
        nc.scalar.dma_start(out=xperm[:], in_=xperm_d[:])
        ad_sb = const.tile([1, NPC], _bf16)
        for j0 in range(0, NPC, TILE_W):
            w = min(TILE_W, NPC - j0)
            ap_ = psa.tile([1, TILE_W], _f32, tag="adps")
            nc.tensor.matmul(
                out=ap_[:, :w],
                lhsT=wad[:],
                rhs=xperm[:, j0 : j0 + w],
                start=True,
                stop=True,
            )
            nc.vector.tensor_copy(out=ad_sb[:, j0 : j0 + w], in_=ap_[:, :w])

        outacc = acc.tile([128, NCOL], _f32)
        sacc = acc.tile([128, NCOL], _f32)

        stage = {}
        for pi, (ta, tb) in enumerate(plan.pairs):
            pos0, n, d = plan.tiles[ta]
            w = n * d
            c0 = int(plan.colstart[ta])
            oc = int(plan.outcol[pi])
            two = tb >= 0
            wtot = 2 * w if two else w

            ci = plan.pair_chunk[pi]
            if ci not in stage:
                plo, phi, clo, chi = plan.chunks[ci]
                st = sb.tile([K_RHS, CHUNK_COLS], _bf16, tag="stage")
                dma_eng = nc.sync
                dma_eng.dma_start(out=st[:, : chi - clo], in_=rhs_d[:, clo:chi])
                stage = {ci: (st, clo)}
            st, clo = stage[ci]
            s0 = c0 - clo
            rt = st[:, s0 : s0 + wtot]

            pmsg = ps.tile([128, TILE_W], _f32, tag="pmsg")
            plog = ps.tile([128, TILE_W], _f32, tag="plog")
            pos0b = plan.tiles[tb][0] if two else 0
            nc.tensor.matmul(
                out=pmsg[0:64, :w], lhsT=lmsg[:], rhs=rt[:, :w], start=True, stop=True
            )
            if two:
                nc.tensor.matmul(
                    out=pmsg[64:128, :w],
                    lhsT=lmsg[:],
                    rhs=rt[:, w : 2 * w],
                    start=True,
                    stop=True,
                )
            nc.tensor.matmul(
                out=plog[0:64, :w], lhsT=llog[:], rhs=rt[:, :w], start=True, stop=False
            )
            if two:
                nc.tensor.matmul(
                    out=plog[64:128, :w],
                    lhsT=llog[:],
                    rhs=rt[:, w : 2 * w],
                    start=True,
                    stop=False,
                )
            nc.tensor.matmul(
                out=plog[0:64, :w],
                lhsT=ones[:],
                rhs=ad_sb[:, pos0 : pos0 + n].unsqueeze(2).broadcast_to([1, n, d]),
                start=False,
                stop=True,
            )
            if two:
                nc.tensor.matmul(
                    out=plog[64:128, :w],
                    lhsT=ones[:],
                    rhs=ad_sb[:, pos0b : pos0b + n]
                    .unsqueeze(2)
                    .broadcast_to([1, n, d]),
                    start=False,
                    stop=True,
                )
            np_ = 128 if two else 64

            pt = sb.tile([128, TILE_W], _bf16, tag="p")
            pt2 = sb.tile([128, TILE_W], _bf16, tag="p2")
            mp = sb.tile([128, TILE_W], _bf16, tag="mp")
            nc.scalar.activation(
                out=mp[:np_, :w],
                in_=pmsg[:np_, :w],
                func=mybir.ActivationFunctionType.Copy,
            )
            nc.scalar.activation(
                out=pt[:np_, :w],
                in_=plog[:np_, :w],
                func=mybir.ActivationFunctionType.Exp,
            )
            nc.scalar.activation(
                out=pt2[:np_, :w],
                in_=plog[:np_, :w],
                func=mybir.ActivationFunctionType.Exp,
                scale=ATT_SLOPE,
            )
            nc.vector.tensor_max(
                out=pt[:np_, :w], in0=pt[:np_, :w], in1=pt2[:np_, :w]
            )
            nc.vector.tensor_mul(out=mp[:np_, :w], in0=mp[:np_, :w], in1=pt[:np_, :w])
            nc.vector.tensor_reduce(
                out=outacc[:np_, oc : oc + n],
                in_=mp[:np_, :w].rearrange("p (n d) -> p n d", d=d),
                axis=mybir.AxisListType.X,
                op=mybir.AluOpType.max,
            )
            nc.vector.tensor_reduce(
                out=sacc[:np_, oc : oc + n],
                in_=pt[:np_, :w].rearrange("p (n d) -> p n d", d=d),
                axis=mybir.AxisListType.X,
                op=mybir.AluOpType.add,
            )
            if not two:
                nc.vector.memset(outacc[64:128, oc : oc + n], 0.0)
                nc.vector.memset(sacc[64:128, oc : oc + n], 1.0)

        rs = acc.tile([128, NCOL], _f32)
        nc.vector.reciprocal_approx_fast(out=rs[:], in_=sacc[:])
        nc.vector.tensor_mul(out=outacc[:], in0=outacc[:], in1=rs[:])
        mask = acc.tile([128, NCOL], _f32)
        nc.vector.scalar_tensor_tensor(
            out=outacc[:],
            in0=outacc[:],
            scalar=float(EMPTY_THR),
            in1=outacc[:],
            op0=mybir.AluOpType.is_ge,
            op1=mybir.AluOpType.mult,
        )
        nc.vector.tensor_scalar(
            out=outacc[:],
            in0=outacc[:],
            scalar1=bvec[:],
            scalar2=None,
            op0=mybir.AluOpType.add,
        )
        nc.vector.scalar_tensor_tensor(
            out=outacc[:],
            in0=outacc[:],
            scalar=ACT_SLOPE,
            in1=outacc[:],
            op0=mybir.AluOpType.mult,
            op1=mybir.AluOpType.max,
        )
        nc.sync.dma_start(out=out_d[:], in_=outacc[:])

    nc.compile()
    return nc


def make_lhs(W, We, a_s, a_e):
    lmsg = np.zeros((K_RHS, DOUT), np.float32)
    lmsg[:DIN] = W
    lmsg[ROW_EA : ROW_EA + DE] = We
    lmsg[ROW_PAD, :] = BIG_NEG
    llog = np.zeros((K_RHS, DOUT), np.float32)
    llog[:DIN] = (W @ a_s)[:, None]
    llog[ROW_EA : ROW_EA + DE] = (We @ a_e)[:, None]
    llog[ROW_PAD, :] = PAD_LOGIT
    return lmsg, llog


def assemble(plan, outs):
    full = np.zeros((N, DOUT), np.float32)
    for pi, (ta, tb) in enumerate(plan.pairs):
        pos0, n, d = plan.tiles[ta]
        oc = int(plan.outcol[pi])
        for c in range(NC):
            nodes = plan.node_map[c, pos0 : pos0 + n]
            full[nodes] = outs[c, 0:64, oc : oc + n].T
            if tb >= 0:
                pos0b, nb, _ = plan.tiles[tb]
                nodesb = plan.node_map[c, pos0b : pos0b + nb]
                full[nodesb] = outs[c, 64:128, oc : oc + n].T
    return full


def kernel(
    X,
    edge_index,
    edge_attr,
    W1,
    We1,
    as1,
    ad1,
    ae1,
    b1,
    W2,
    We2,
    as2,
    ad2,
    ae2,
    b2,
):
    trace = os.environ.get("GAT_TRACE") == "1"
    if trace:
        _install_ntff_shim()
    LAST_EXEC_NS.clear()
    X = np.asarray(X, np.float32)
    edge_attr = np.asarray(edge_attr, np.float32)
    src = np.asarray(edge_index[0], np.int64)
    dst = np.asarray(edge_index[1], np.int64)
    W1, We1, as1, ad1, ae1, b1 = [
        np.asarray(a, np.float32) for a in (W1, We1, as1, ad1, ae1, b1)
    ]
    W2, We2, as2, ad2, ae2, b2 = [
        np.asarray(a, np.float32) for a in (W2, We2, as2, ad2, ae2, b2)
    ]

    plan = make_plan(dst)
    slot_src, slot_eid = make_slot_maps(plan, src, dst)

    valid_e = slot_eid >= 0
    ea = edge_attr[np.where(valid_e, slot_eid, 0)]
    ea[~valid_e] = 0.0
    ea_part = np.zeros((NC, DE + 1, plan.S), np.float32)
    ea_part[:, :DE, :] = ea.transpose(0, 2, 1)
    ea_part[:, DE, :] = (~valid_e).astype(np.float32)
    del ea

    nc_prog = build_program(plan)

    valid_s = slot_src >= 0

    def layer(node_feat, W, We, a_s, a_e, a_d, b):
        rhs = np.zeros((NC, K_RHS, plan.S), np.float32)
        xs = node_feat[np.where(valid_s, slot_src, 0)]
        xs[~valid_s] = 0.0
        rhs[:, :DIN, :] = xs.transpose(0, 2, 1)
        rhs[:, ROW_EA : ROW_EA + DE + 1, :] = ea_part
        xperm = node_feat[plan.node_map].transpose(0, 2, 1)
        lmsg, llog = make_lhs(W, We, a_s, a_e)
        wad = (W @ a_d)[:, None]
        bvec = np.concatenate([b, b]).reshape(128, 1).astype(np.float32)
        rhs16, xperm16 = _bf(rhs), np.ascontiguousarray(_bf(xperm))
        in_maps = [
            {
                "rhs": rhs16[c],
                "xperm": xperm16[c],
                "lmsg": _bf(lmsg),
                "llog": _bf(llog),
                "wad": _bf(wad),
                "bvec": bvec,
                "ones": np.ones((1, DOUT), ml_dtypes.bfloat16),
            }
            for c in range(NC)
        ]
        res = run_bass_kernel_spmd(
            nc_prog, in_maps, core_ids=list(range(NC)), trace=trace
        )
        if trace and res.exec_time_ns:
            LAST_EXEC_NS.append(res.exec_time_ns)
        outs = np.stack([res.results[c]["out"] for c in range(NC)])
        return assemble(plan, outs)

    c1 = layer(X, W1, We1, as1, ae1, ad1, b1)
    c2 = layer(c1, W2, We2, as2, ae2, ad2, b2)
    return c2
